# revision 19
# baseline (speedup 1.0000x reference)
"""Trainium2 Bass kernel for nn_LlamaAttention_cam (sparse attention + CaM merge).

Sharding: tensor-parallel over heads across 8 NeuronCores (2 heads/core).
Each core computes its heads' QKV projections, RoPE, masked attention
(start+recent keep mask), CaM rank-1 correction for the last chunk, and a
partial o_proj.  The host sums the 8 partial outputs (the reduction of the
head-parallel o_proj), replacing the all-reduce.

Token positions are permuted host-side to [start | recent | rest] so the
1023 kept keys occupy the first 1023 columns: key blocks are 8 full
128-blocks (block 7 has one non-key token, masked via an indicator
stationary + a zeroed V row).  The CaM chunk (t in [1792,2048)) lands in
t-block 1 at columns 767..1022, so the serial CaM chain overlaps the rest
of the pipeline instead of sitting in the tail.  hs streams in two waves
(kept tokens first) so K/V projections start early.  o_proj is software-
pipelined one t-block behind attention.  All matmul IO is fp16; PSUM
accumulation fp32; softmax/CaM scalar math fp32.
"""

import sys

for _p in ("/opt/trn_rl_repo",):
    if _p not in sys.path:
        sys.path.append(_p)

import numpy as np

import concourse.bass as bass
import concourse.mybir as mybir
import concourse.tile as tile
from concourse import bacc, bass_utils

F32 = mybir.dt.float32
F16 = mybir.dt.float16
AF = mybir.ActivationFunctionType

T = 2048
DM = 2048
H = 16
D = 128
NCORES = 8
HL = H // NCORES          # heads per core = 2
JC = HL * D               # local attn width = 256
SB = 204                  # start keep
RB = 819                  # recent keep
KC = DM // 128            # 16 model-dim chunks
TB = T // 512             # 4 t-blocks of 512
NB = 8                    # kept-key blocks (perm cols 0..1024, col 1023 masked)
# perm order: [0..204) + [1229..2048) + [204..1229)
# CaM: evict key 1229 -> perm col 204 (block 1, row 76)
# CaM q range t in [1792,2048) -> perm cols 767..1022 = tb1 local cols 255..510
CAMTB = 1


def _build_nc():
    nc = bacc.Bacc("TRN2", target_bir_lowering=False, debug=False,
                   num_devices=NCORES)
    hsT = nc.dram_tensor("hsT", [DM, T], F16, kind="ExternalInput").ap()
    wq = nc.dram_tensor("wq", [128, KC * JC], F16, kind="ExternalInput").ap()
    wk = nc.dram_tensor("wk", [128, KC * JC], F16, kind="ExternalInput").ap()
    wv = nc.dram_tensor("wv", [128, KC * JC], F16, kind="ExternalInput").ap()
    wo = nc.dram_tensor("wo", [128, HL * DM], F16, kind="ExternalInput").ap()
    cosT = nc.dram_tensor("cosT", [D, T], F32, kind="ExternalInput").ap()
    sinTs = nc.dram_tensor("sinTs", [D, T], F32, kind="ExternalInput").ap()
    protT = nc.dram_tensor("protT", [D, D],
                           mybir.dt.float32r, kind="ExternalInput").ap()
    u2 = nc.dram_tensor("u2", [1, HL], F32, kind="ExternalInput").ap()
    masksh = nc.dram_tensor("masksh", [128, 2], F16, kind="ExternalInput").ap()
    masksf = nc.dram_tensor("masksf", [128, 1], F32, kind="ExternalInput").ap()
    po = nc.dram_tensor("po", [T, DM], F16, kind="ExternalOutput").ap()

    with tile.TileContext(nc) as tc:
        with (
            tc.tile_pool(name="resid", bufs=1) as pres,        # long-lived
            tc.tile_pool(name="e16", bufs=12) as pe16,         # f16 [128,512] E tiles
            tc.tile_pool(name="tmp", bufs=8) as ptmp,          # f32 [128,512] transients
            tc.tile_pool(name="post", bufs=3) as ppost,        # f16 [128,2048] out staging
            tc.tile_pool(name="rows", bufs=4) as prow,         # small [1,*] tiles
            tc.tile_pool(name="ps", bufs=4, space="PSUM") as pps,
            tc.tile_pool(name="psav", bufs=2, space="PSUM") as pav,
            tc.tile_pool(name="psdn", bufs=2, space="PSUM") as ppsd,
        ):
            # ---- input DMAs: wk first, then kept-token hs wave, then the rest ----
            wk_sb = pres.tile([128, KC * JC], F16, tag="wk")
            hs_sb = [pres.tile([128, T], F16, tag=f"hs{kc}", name=f"hs{kc}")
                     for kc in range(KC)]
            wv_sb = pres.tile([128, KC * JC], F16, tag="wv")
            nc.sync.dma_start(wk_sb[:, 0:512], wk[:, 0:512])
            nc.sync.dma_start(hs_sb[0][:, 0:1024], hsT[0:128, 0:1024])
            nc.sync.dma_start(wk_sb[:, 512:2048], wk[:, 512:2048])
            nc.sync.dma_start(wk_sb[:, 2048:4096], wk[:, 2048:4096])
            for kc in range(1, 8):
                nc.sync.dma_start(hs_sb[kc][:, 0:1024],
                                  hsT[kc * 128:(kc + 1) * 128, 0:1024])
            nc.sync.dma_start(wv_sb[:], wv[:])
            for kc in range(8, KC):
                nc.sync.dma_start(hs_sb[kc][:, 0:1024],
                                  hsT[kc * 128:(kc + 1) * 128, 0:1024])
            cosT_sb = pres.tile([D, T], F32, tag="cos")
            sinTs_sb = pres.tile([D, T], F32, tag="sin")
            protT_sb = pres.tile([D, D], mybir.dt.float32r, tag="prot")
            nc.sync.dma_start(cosT_sb[:], cosT[:])
            nc.sync.dma_start(sinTs_sb[:], sinTs[:])
            nc.sync.dma_start(protT_sb[:], protT[:])
            wq_sb = pres.tile([128, KC * JC], F16, tag="wq")
            nc.sync.dma_start(wq_sb[:], wq[:])
            for kc in range(KC):
                nc.sync.dma_start(hs_sb[kc][:, 1024:2048],
                                  hsT[kc * 128:(kc + 1) * 128, 1024:2048])
            wo_sb = pres.tile([128, HL * DM], F16, tag="wo")
            nc.sync.dma_start(wo_sb[:], wo[:])
            u2_sb = pres.tile([1, HL], F32, tag="u2")
            nc.sync.dma_start(u2_sb[:], u2[:])
            masksh_sb = pres.tile([128, 2], F16, tag="masksh")
            nc.sync.dma_start(masksh_sb[:], masksh[:])
            masksf_sb = pres.tile([128, 1], F32, tag="masksf")
            nc.sync.dma_start(masksf_sb[:], masksf[:])
            maskB16 = masksh_sb[:, 0:1]    # rows <= 126 (drop key col 1023)
            mask77A = masksh_sb[:, 1:2]    # rows <= 76 (start + evict rows of blk 1)

            ones_f = pres.tile([128, 1], F32, tag="ones_f")
            nc.vector.memset(ones_f[:], 1.0)
            ones = pres.tile([128, 1], F16, tag="ones")
            nc.vector.tensor_copy(ones[:], ones_f[:])

            # rope'd q/k in [d, t] layout; v in [t, d_local] layout
            qrT = [pres.tile([D, T], F16, tag=f"qrT{l}", name=f"qrT{l}") for l in range(HL)]
            krT = [pres.tile([D, T], F16, tag=f"krT{l}", name=f"krT{l}") for l in range(HL)]
            vt = [pres.tile([128, JC], F16, tag=f"vt{b}", name=f"vt{b}") for b in range(NB)]
            outT = [pres.tile([D, T], F16, tag=f"outT{l}", name=f"outT{l}") for l in range(HL)]

            F32R = mybir.dt.float32r

            def rope_pre(ps_ap, w):
                raw = ptmp.tile([128, w], F32R, tag="tmp")
                nc.scalar.copy(raw[:], ps_ap)
                return raw

            def rope_rot(raw, pool=None, tag="av"):
                rot = (pool or pav).tile([128, raw.shape[1]], F32, tag=tag)
                nc.tensor.matmul(rot[:], protT_sb[:], raw[:],
                                 start=True, stop=True)
                return rot

            def rope_post(raw, rot, dst_ap, c0, c1):
                w = c1 - c0
                t1 = ptmp.tile([128, w], F32, tag="tmp")
                nc.gpsimd.tensor_mul(t1[:], raw[:].bitcast(F32), cosT_sb[:, c0:c1])
                t2 = ptmp.tile([128, w], F32, tag="tmp")
                nc.vector.tensor_mul(t2[:], rot[:], sinTs_sb[:, c0:c1])
                nc.vector.tensor_add(dst_ap, t1[:], t2[:])

            # ---------------- phase K: kept keys (perm cols 0..1024), kc-outer ----
            psk = []
            for l in range(HL):
                pska = pps.tile([128, 512], F32, tag="ps", name=f"pska{l}")
                pskb = pps.tile([128, 512], F32, tag="ps", name=f"pskb{l}")
                psk.append((pska, pskb))
            for kc in range(KC):
                for l in range(HL):
                    wsl = wk_sb[:, kc * JC + l * 128: kc * JC + (l + 1) * 128]
                    nc.tensor.matmul(psk[l][0][:], wsl, hs_sb[kc][:, 0:512],
                                     start=(kc == 0), stop=(kc == KC - 1))
                    nc.tensor.matmul(psk[l][1][:], wsl, hs_sb[kc][:, 512:1024],
                                     start=(kc == 0), stop=(kc == KC - 1))
            kraw = []
            for l in range(HL):
                kraw.append((rope_pre(psk[l][0][:], 512),
                             rope_pre(psk[l][1][:], 512)))

            # ---------------- phase V (paired blocks, issued after q-tb0) ----------
            def v_phase():
                for bp in range(0, NB, 2):
                    if bp == 2:
                        for l in range(HL):
                            ra, rb = kraw[l]
                            rota = rope_rot(ra)
                            rope_post(ra, rota, krT[l][:, 0:512], 0, 512)
                            rotb = rope_rot(rb)
                            rope_post(rb, rotb, krT[l][:, 512:1024], 512, 1024)
                    psvA = pps.tile([128, JC], F32, tag="ps", name=f"psvA{bp}")
                    psvB = pps.tile([128, JC], F32, tag="ps", name=f"psvB{bp}")
                    for kc in range(KC):
                        nc.tensor.matmul(psvA[:], hs_sb[kc][:, bp * 128:(bp + 1) * 128],
                                         wv_sb[:, kc * JC:(kc + 1) * JC],
                                         start=(kc == 0), stop=(kc == KC - 1))
                        nc.tensor.matmul(psvB[:],
                                         hs_sb[kc][:, (bp + 1) * 128:(bp + 2) * 128],
                                         wv_sb[:, kc * JC:(kc + 1) * JC],
                                         start=(kc == 0), stop=(kc == KC - 1))
                    nc.scalar.copy(vt[bp][:], psvA[:])
                    nc.vector.tensor_copy(vt[bp + 1][:], psvB[:])
                    if bp + 1 == NB - 1:
                        # zero the non-key row (perm col 1023) so AV ignores it
                        nc.vector.tensor_scalar_mul(vt[bp + 1][:], vt[bp + 1][:],
                                                    masksf_sb[:, 0:1])

            # ---------------- phase Q + attention + pipelined o_proj ----------------
            def o_proj(tb):
                for j in range(4):
                    ti = tb * 4 + j
                    last = (ti == 4 * TB - 1)
                    postg = ppost.tile([128, DM], F16, tag="post")
                    for mb in range(TB):
                        pso = pps.tile([128, 512], F32, tag="ps")
                        for l in range(HL):
                            nc.tensor.matmul(
                                pso[:], outT[l][:, ti * 128:(ti + 1) * 128],
                                wo_sb[:, l * DM + mb * 512: l * DM + (mb + 1) * 512],
                                start=(l == 0), stop=(l == HL - 1))
                        if mb % 2 == 0:
                            nc.scalar.copy(postg[:, mb * 512:(mb + 1) * 512], pso[:])
                        else:
                            nc.vector.tensor_copy(postg[:, mb * 512:(mb + 1) * 512],
                                                  pso[:])
                        if last:
                            nc.sync.dma_start(
                                po[ti * 128:(ti + 1) * 128,
                                   mb * 512:(mb + 1) * 512],
                                postg[:, mb * 512:(mb + 1) * 512])
                    if not last:
                        nc.sync.dma_start(po[ti * 128:(ti + 1) * 128, :], postg[:])

            v_phase()
            pend = {"fin": None}

            for tb in range(TB):
                ts5 = slice(tb * 512, tb * 512 + 512)
                qraw = []
                for l in range(HL):
                    psq = pps.tile([128, 512], F32, tag="ps")
                    for kc in range(KC):
                        nc.tensor.matmul(
                            psq[:], wq_sb[:, kc * JC + l * 128: kc * JC + (l + 1) * 128],
                            hs_sb[kc][:, ts5], start=(kc == 0), stop=(kc == KC - 1))
                    qraw.append(rope_pre(psq[:], 512))
                if pend["fin"] is not None:
                    pend["fin"]()
                    pend["fin"] = None
                for l in range(HL):
                    rot = rope_rot(qraw[l], pool=pps, tag="ps")
                    rope_post(qraw[l], rot, qrT[l][:, ts5],
                              tb * 512, tb * 512 + 512)
                if tb > 0:
                    o_proj(tb - 1)

                cam = []  # per-head deferred CaM state
                psavl = []
                recipl = []
                for l in range(HL):
                    E = []
                    for b in range(NB):
                        pst = pps.tile([128, 512], F32, tag="ps")
                        nc.tensor.matmul(pst[:], krT[l][:, b * 128:(b + 1) * 128],
                                         qrT[l][:, ts5], start=True, stop=True)
                        e = pe16.tile([128, 512], F16, tag="e")
                        nc.scalar.activation(e[:], pst[:], AF.Exp)
                        E.append(e)
                    psav = pav.tile([128, 512], F32, tag="av")
                    psdn = ppsd.tile([1, 512], F32, tag="dn")
                    for b in range(NB):
                        dstat = maskB16 if b == NB - 1 else ones[:]
                        nc.tensor.matmul(psav[:], vt[b][:, l * D:(l + 1) * D], E[b][:],
                                         start=(b == 0), stop=(b == NB - 1))
                        nc.tensor.matmul(psdn[:], dstat, E[b][:],
                                         start=(b == 0), stop=(b == NB - 1))
                    dn_sb = prow.tile([1, 512], F32, tag="row512")
                    nc.vector.tensor_copy(dn_sb[:], psdn[:])
                    recip = prow.tile([1, 512], F32, tag="row512")
                    nc.vector.reciprocal(recip[:], dn_sb[:])
                    rbf = ptmp.tile([128, 512], F32, tag="tmp")
                    nc.gpsimd.partition_broadcast(rbf[:], recip[:])
                    if tb != CAMTB:
                        nc.vector.tensor_mul(outT[l][:, ts5], psav[:], rbf[:])
                    psavl.append(psav)
                    recipl.append(rbf)

                    if tb == CAMTB:
                        # ---- CaM scalar chain (PE part deferred past attn l1) ----
                        # sum of E over start+evict rows at CaM cols (255..511)
                        pssA = ppsd.tile([1, 256], F32, tag="dn")
                        nc.tensor.matmul(pssA[:], ones[:], E[0][:, 255:511],
                                         start=True, stop=False)
                        nc.tensor.matmul(pssA[:], mask77A, E[1][:, 255:511],
                                         start=False, stop=True)
                        pssA_sb = prow.tile([1, 256], F32, tag="row256")
                        nc.scalar.copy(pssA_sb[:], pssA[:])
                        # E row of evict key (perm col 204 = block 1 row 76)
                        erow16 = prow.tile([1, 256], F16, tag="row256h")
                        nc.gpsimd.dma_start(erow16[:], E[1][76:77, 255:511])
                        erow = prow.tile([1, 256], F32, tag="row256")
                        nc.vector.tensor_copy(erow[:], erow16[:])
                        srec = prow.tile([1, 256], F32, tag="row256")
                        nc.vector.tensor_sub(srec[:], psdn[0:1, 255:511], pssA_sb[:])
                        # scalars at t = 2047 (perm col 1022 = within-slice 255)
                        r_last = recip[0:1, 510:511]
                        num = prow.tile([1, 1], F32, tag="sc")
                        nc.vector.tensor_mul(num[:], erow[0:1, 255:256], r_last)
                        mean = prow.tile([1, 1], F32, tag="sc")
                        nc.vector.tensor_mul(mean[:], srec[0:1, 255:256], r_last)
                        nc.vector.tensor_scalar_mul(mean[:], mean[:], 1.0 / 818.0)
                        nc.vector.tensor_scalar_add(mean[:], mean[:], 1e-6)
                        um = prow.tile([1, 1], F32, tag="sc")
                        nc.vector.tensor_mul(um[:], u2_sb[0:1, l:l + 1], mean[:])
                        bern = prow.tile([1, 1], F32, tag="sc")
                        nc.vector.tensor_tensor(bern[:], um[:], num[:],
                                                mybir.AluOpType.is_lt)
                        bs = prow.tile([1, 1], F32, tag="sc")
                        nc.vector.tensor_scalar_mul(bs[:], bern[:], 1.0 / RB)
                        coef_f = prow.tile([1, 256], F32, tag="row256")
                        nc.vector.tensor_scalar_mul(coef_f[:], srec[:], bs[:])
                        coef = prow.tile([1, 256], F16, tag="row256h")
                        nc.vector.tensor_copy(coef[:], coef_f[:])
                        vrow = prow.tile([1, D], F16, tag="vrow")
                        nc.gpsimd.dma_start(vrow[:], vt[1][76:77, l * D:(l + 1) * D])
                        cam.append((coef, vrow))

                # normalize; at tb==CAMTB defer (incl. CaM rank-1) into next tb's
                # slot so the CaM scalar chain never head-blocks the PE queue
                if tb == CAMTB:
                    def fin(ts5=ts5, cam=cam, psavl=psavl, recipl=recipl):
                        for l in range(HL):
                            psav, rbf = psavl[l], recipl[l]
                            coef, vrow = cam[l]
                            pscr = pps.tile([128, 256], F32, tag="ps")
                            nc.tensor.matmul(pscr[:], vrow[:], coef[:],
                                             start=True, stop=True)
                            nout = ptmp.tile([128, 512], F32, tag="tmp")
                            nc.vector.tensor_mul(nout[:], psav[:], rbf[:])
                            corr = ptmp.tile([128, 256], F32, tag="tmp")
                            nc.vector.tensor_mul(corr[:], pscr[:], rbf[:, 255:511])
                            nc.vector.tensor_add(nout[:, 255:511],
                                                 nout[:, 255:511], corr[:])
                            nc.vector.tensor_copy(outT[l][:, ts5], nout[:])
                    pend["fin"] = fin


            o_proj(TB - 1)

    nc.compile()
    return nc


_NC_CACHE = None


def _get_nc():
    global _NC_CACHE
    if _NC_CACHE is None:
        _NC_CACHE = _build_nc()
    return _NC_CACHE


PERM = np.concatenate([np.arange(0, SB), np.arange(T - RB, T),
                       np.arange(SB, T - RB)])


def make_in_maps(hidden_states, Wq, Wk, Wv, Wo):
    hs = np.asarray(hidden_states, np.float32).reshape(T, DM)
    hs = np.nan_to_num(hs, nan=0.0, posinf=1e4, neginf=-1e4)
    hsT = np.ascontiguousarray(hs.T[:, PERM].astype(np.float16))
    Wq = np.asarray(Wq, np.float32)
    Wk = np.asarray(Wk, np.float32)
    Wv = np.asarray(Wv, np.float32)
    Wo = np.asarray(Wo, np.float32)

    inv_freq = 1.0 / (10000.0 ** (np.arange(0, D, 2, dtype=np.float32) / D))
    freqs = np.arange(T, dtype=np.float32)[:, None] * inv_freq[None, :]
    emb = np.concatenate([freqs, freqs], axis=-1)          # [T, D]
    cosT = np.ascontiguousarray(np.cos(emb).T[:, PERM].astype(np.float32))
    sinTs = np.ascontiguousarray(np.sin(emb).T[:, PERM].astype(np.float32))
    # rotate-half as a PE stationary: rot(x)[i] = -x[i+64] (i<64), x[i-64] (else)
    prot = np.zeros((D, D), np.float32)
    for i in range(64):
        prot[i + 64, i] = -1.0
        prot[i, i + 64] = 1.0

    import jax
    import jax.numpy as jnp
    u_full = np.asarray(
        jax.random.uniform(jax.random.key(42), (1, H), jnp.float32))

    maskh = np.zeros((128, 2), np.float16)
    maskh[:127, 0] = 1.0      # maskB16: drop row 127 of key block 7
    maskh[:77, 1] = 1.0       # mask77A: start+evict rows of key block 1
    maskf = np.zeros((128, 1), np.float32)
    maskf[:127, 0] = 1.0

    scale = 1.0 / np.sqrt(np.float32(D))

    def wlayout(wT):
        # wT: [DM, JC] -> SBUF layout [128, KC*JC]: [p, kc*JC + j]
        return np.ascontiguousarray(
            wT.reshape(KC, 128, JC).transpose(1, 0, 2).reshape(128, KC * JC)
            .astype(np.float16))

    in_maps = []
    for c in range(NCORES):
        js = slice(c * JC, (c + 1) * JC)
        woT = Wo[:, js].T                                  # [JC, DM]
        wo_l = np.ascontiguousarray(
            woT.reshape(HL, 128, DM).transpose(1, 0, 2).reshape(128, HL * DM)
            .astype(np.float16))
        in_maps.append({
            "hsT": hsT,
            "wq": wlayout(Wq[js, :].T * scale),
            "wk": wlayout(Wk[js, :].T),
            "wv": wlayout(Wv[js, :].T),
            "wo": wo_l,
            "cosT": cosT,
            "sinTs": sinTs,
            "protT": prot,
            "u2": np.ascontiguousarray(u_full[:, c * HL:(c + 1) * HL]),
            "masksh": maskh,
            "masksf": maskf,
        })
    return in_maps


def kernel(hidden_states, Wq, Wk, Wv, Wo):
    nc = _get_nc()
    in_maps = make_in_maps(hidden_states, Wq, Wk, Wv, Wo)
    res = bass_utils.run_bass_kernel_spmd(nc, in_maps,
                                          core_ids=list(range(NCORES)))
    acc = np.zeros((T, DM), np.float32)
    for c in range(NCORES):
        acc += res.results[c]["po"].astype(np.float32)
    out = np.empty((T, DM), np.float32)
    out[PERM] = acc                                       # undo token permutation
    out = np.nan_to_num(out, nan=0.0, posinf=1e4, neginf=-1e4)
    return out.reshape(1, T, DM)


# revision 29
# speedup vs baseline: 1.1446x; 1.1446x over previous
"""Trainium2 Bass kernel for nn_LlamaAttention_cam (sparse attention + CaM merge).

Sharding: tensor-parallel over heads across 8 NeuronCores (2 heads/core).
Each core computes its heads' QKV projections, RoPE, masked attention
(start+recent keep mask), CaM rank-1 correction for the last chunk, and a
partial o_proj.  The host sums the 8 partial outputs (the reduction of the
head-parallel o_proj), replacing the all-reduce.

Token positions are permuted host-side to [start | recent | rest] so the
1023 kept keys occupy the first 1023 columns: key blocks are 8 full
128-blocks (block 7 has one non-key token, masked via an indicator
stationary + a zeroed V row).  The CaM chunk (t in [1792,2048)) lands in
t-block 1 at columns 767..1022, so the serial CaM chain overlaps the rest
of the pipeline instead of sitting in the tail.  hs streams in two waves
(kept tokens first) so K/V projections start early.  o_proj is software-
pipelined one t-block behind attention.  All matmul IO is fp16; PSUM
accumulation fp32; softmax/CaM scalar math fp32.
"""

import sys

for _p in ("/opt/trn_rl_repo",):
    if _p not in sys.path:
        sys.path.append(_p)

import numpy as np

import concourse.bass as bass
import concourse.bass_isa as bass_isa
import concourse.mybir as mybir
import concourse.tile as tile
from concourse import bacc, bass_utils

F32 = mybir.dt.float32
F16 = mybir.dt.float16
AF = mybir.ActivationFunctionType

T = 2048
DM = 2048
H = 16
D = 128
NCORES = 8
HL = H // NCORES          # heads per core = 2
JC = HL * D               # local attn width = 256
SB = 204                  # start keep
RB = 819                  # recent keep
KC = DM // 128            # 16 model-dim chunks
TB = T // 512             # 4 t-blocks of 512
TBS = [(0, 512), (512, 1024), (1024, 1536), (1536, 2048)]
NB = 8                    # kept-key blocks (perm cols 0..1024, col 1023 masked)
# perm order: [0..204) + [1229..2048) + [204..1229)
# CaM: evict key 1229 -> perm col 204 (block 1, row 76)
# CaM q range t in [1792,2048) -> perm cols 767..1022 = tb1 local cols 255..510
CAMTB = 1


def _build_nc():
    nc = bacc.Bacc("TRN2", target_bir_lowering=False, debug=False,
                   num_devices=NCORES)
    hsT = nc.dram_tensor("hsT", [DM, T], F16, kind="ExternalInput").ap()
    wq = nc.dram_tensor("wq", [128, KC * JC], F16, kind="ExternalInput").ap()
    wk = nc.dram_tensor("wk", [128, KC * JC], F16, kind="ExternalInput").ap()
    wv = nc.dram_tensor("wv", [128, KC * JC], F16, kind="ExternalInput").ap()
    wo = nc.dram_tensor("wo", [128, HL * DM], F16, kind="ExternalInput").ap()
    cosT = nc.dram_tensor("cosT", [D, T], F32, kind="ExternalInput").ap()
    sinTs = nc.dram_tensor("sinTs", [D, T], F32, kind="ExternalInput").ap()
    protT = nc.dram_tensor("protT", [D, D],
                           mybir.dt.float32r, kind="ExternalInput").ap()
    u2 = nc.dram_tensor("u2", [1, HL], F32, kind="ExternalInput").ap()
    masksh = nc.dram_tensor("masksh", [128, 2], F16, kind="ExternalInput").ap()
    masksf = nc.dram_tensor("masksf", [128, 1], F32, kind="ExternalInput").ap()
    po = nc.dram_tensor("po", [T, DM], F16, kind="ExternalOutput").ap()

    with tile.TileContext(nc) as tc:
        with (
            tc.tile_pool(name="resid", bufs=1) as pres,        # long-lived
            tc.tile_pool(name="e16", bufs=12) as pe16,         # f16 [128,512] E tiles
            tc.tile_pool(name="tmp", bufs=8) as ptmp,          # f32 [128,512] transients
            tc.tile_pool(name="post", bufs=3) as ppost,        # f16 [128,2048] out staging
            tc.tile_pool(name="rows", bufs=4) as prow,         # small [1,*] tiles
            tc.tile_pool(name="ps", bufs=4, space="PSUM") as pps,
            tc.tile_pool(name="psav", bufs=2, space="PSUM") as pav,
            tc.tile_pool(name="psdn", bufs=2, space="PSUM") as ppsd,
        ):
            # ---- input DMAs: wk first, then kept-token hs wave, then the rest ----
            wk_sb = pres.tile([128, KC * JC], F16, tag="wk")
            hs_sb = [pres.tile([128, T], F16, tag=f"hs{kc}", name=f"hs{kc}")
                     for kc in range(KC)]
            wv_sb = pres.tile([128, KC * JC], F16, tag="wv")
            nc.sync.dma_start(wk_sb[:, 0:512], wk[:, 0:512])
            nc.sync.dma_start(hs_sb[0][:, 0:1024], hsT[0:128, 0:1024])
            nc.sync.dma_start(hs_sb[1][:, 0:1024], hsT[128:256, 0:1024])
            nc.sync.dma_start(wk_sb[:, 512:1024], wk[:, 512:1024])
            for kc in range(2, 4):
                nc.sync.dma_start(hs_sb[kc][:, 0:1024],
                                  hsT[kc * 128:(kc + 1) * 128, 0:1024])
            nc.sync.dma_start(wk_sb[:, 1024:2048], wk[:, 1024:2048])
            for kc in range(4, 8):
                nc.sync.dma_start(hs_sb[kc][:, 0:1024],
                                  hsT[kc * 128:(kc + 1) * 128, 0:1024])
            nc.sync.dma_start(wk_sb[:, 2048:4096], wk[:, 2048:4096])
            for kc in range(8, KC):
                nc.sync.dma_start(hs_sb[kc][:, 0:1024],
                                  hsT[kc * 128:(kc + 1) * 128, 0:1024])
            for qq in range(4):
                nc.sync.dma_start(wv_sb[:, qq * 1024:(qq + 1) * 1024],
                                  wv[:, qq * 1024:(qq + 1) * 1024])
            cosT_sb = pres.tile([D, T], F32, tag="cos")
            sinTs_sb = pres.tile([D, T], F32, tag="sin")
            protT_sb = pres.tile([D, D], mybir.dt.float32r, tag="prot")
            nc.sync.dma_start(cosT_sb[:], cosT[:])
            nc.sync.dma_start(sinTs_sb[:], sinTs[:])
            nc.sync.dma_start(protT_sb[:], protT[:])
            wq_sb = pres.tile([128, KC * JC], F16, tag="wq")
            nc.sync.dma_start(wq_sb[:], wq[:])
            for kc in range(KC):
                nc.sync.dma_start(hs_sb[kc][:, 1024:2048],
                                  hsT[kc * 128:(kc + 1) * 128, 1024:2048])
            wo_sb = pres.tile([128, HL * DM], F16, tag="wo")
            nc.sync.dma_start(wo_sb[:], wo[:])
            u2_sb = pres.tile([1, HL], F32, tag="u2")
            nc.sync.dma_start(u2_sb[:], u2[:])
            masksh_sb = pres.tile([128, 2], F16, tag="masksh")
            nc.sync.dma_start(masksh_sb[:], masksh[:])
            masksf_sb = pres.tile([128, 1], F32, tag="masksf")
            nc.sync.dma_start(masksf_sb[:], masksf[:])
            maskB16 = masksh_sb[:, 0:1]    # rows <= 126 (drop key col 1023)
            mask77A = masksh_sb[:, 1:2]    # rows <= 76 (start + evict rows of blk 1)

            ones_f = pres.tile([128, 1], F32, tag="ones_f")
            nc.vector.memset(ones_f[:], 1.0)
            ones = pres.tile([128, 1], F16, tag="ones")
            nc.vector.tensor_copy(ones[:], ones_f[:])

            # rope'd q/k in [d, t] layout; v in [t, d_local] layout
            qrT = [pres.tile([D, T], F16, tag=f"qrT{l}", name=f"qrT{l}") for l in range(HL)]
            krT = [pres.tile([D, T], F16, tag=f"krT{l}", name=f"krT{l}") for l in range(HL)]
            vt = [pres.tile([128, JC], F16, tag=f"vt{b}", name=f"vt{b}") for b in range(NB)]
            outT = [pres.tile([D, T], F16, tag=f"outT{l}", name=f"outT{l}") for l in range(HL)]

            F32R = mybir.dt.float32r

            def rope_pre(ps_ap, w):
                raw = ptmp.tile([128, w], F32R, tag="tmp")
                nc.scalar.copy(raw[:], ps_ap)
                return raw

            def rope_rot(raw, pool=None, tag="av"):
                rot = (pool or pav).tile([128, raw.shape[1]], F32, tag=tag)
                nc.tensor.matmul(rot[:], protT_sb[:], raw[:],
                                 start=True, stop=True)
                return rot

            def rope_post(raw, rot, dst_ap, c0, c1):
                w = c1 - c0
                t1 = ptmp.tile([128, w], F32, tag="tmp")
                nc.gpsimd.tensor_mul(t1[:], raw[:].bitcast(F32), cosT_sb[:, c0:c1])
                t2 = ptmp.tile([128, w], F32, tag="tmp")
                nc.vector.tensor_mul(t2[:], rot[:], sinTs_sb[:, c0:c1])
                nc.vector.tensor_add(dst_ap, t1[:], t2[:])

            # ---------------- phase K: kept keys (perm cols 0..1024), kc-outer ----
            psk = []
            for l in range(HL):
                pska = pps.tile([128, 512], F32, tag="ps", name=f"pska{l}")
                pskb = pps.tile([128, 512], F32, tag="ps", name=f"pskb{l}")
                psk.append((pska, pskb))
            for kc in range(KC):
                for l in range(HL):
                    wsl = wk_sb[:, kc * JC + l * 128: kc * JC + (l + 1) * 128]
                    nc.tensor.matmul(psk[l][0][:], wsl, hs_sb[kc][:, 0:512],
                                     start=(kc == 0), stop=(kc == KC - 1))
                    nc.tensor.matmul(psk[l][1][:], wsl, hs_sb[kc][:, 512:1024],
                                     start=(kc == 0), stop=(kc == KC - 1))
            kraw = []
            for l in range(HL):
                kraw.append((rope_pre(psk[l][0][:], 512),
                             rope_pre(psk[l][1][:], 512)))

            # ---------------- phase V (paired blocks, issued after q-tb0) ----------
            def v_phase():
                for bp in range(0, NB, 2):
                    if bp == 2:
                        for l in range(HL):
                            ra, rb = kraw[l]
                            rota = rope_rot(ra)
                            rope_post(ra, rota, krT[l][:, 0:512], 0, 512)
                            rotb = rope_rot(rb)
                            rope_post(rb, rotb, krT[l][:, 512:1024], 512, 1024)
                    psvA = pps.tile([128, JC], F32, tag="ps", name=f"psvA{bp}")
                    psvB = pps.tile([128, JC], F32, tag="ps", name=f"psvB{bp}")
                    for kc in range(KC):
                        nc.tensor.matmul(psvA[:], hs_sb[kc][:, bp * 128:(bp + 1) * 128],
                                         wv_sb[:, kc * JC:(kc + 1) * JC],
                                         start=(kc == 0), stop=(kc == KC - 1))
                        nc.tensor.matmul(psvB[:],
                                         hs_sb[kc][:, (bp + 1) * 128:(bp + 2) * 128],
                                         wv_sb[:, kc * JC:(kc + 1) * JC],
                                         start=(kc == 0), stop=(kc == KC - 1))
                    nc.scalar.copy(vt[bp][:], psvA[:])
                    nc.vector.tensor_copy(vt[bp + 1][:], psvB[:])
                    if bp + 1 == NB - 1:
                        # zero the non-key row (perm col 1023) so AV ignores it
                        nc.vector.tensor_scalar_mul(vt[bp + 1][:], vt[bp + 1][:],
                                                    masksf_sb[:, 0:1])

            # ---------------- phase Q + attention + pipelined o_proj ----------------
            def o_proj(tb):
                c0, c1 = TBS[tb]
                for ti in range(c0 // 128, c1 // 128):
                    last = (ti == 4 * TB - 1)
                    postg = ppost.tile([128, DM], F16, tag="post")
                    for mb in range(TB):
                        pso = pps.tile([128, 512], F32, tag="ps")
                        for l in range(HL):
                            nc.tensor.matmul(
                                pso[:], outT[l][:, ti * 128:(ti + 1) * 128],
                                wo_sb[:, l * DM + mb * 512: l * DM + (mb + 1) * 512],
                                start=(l == 0), stop=(l == HL - 1))
                        if mb % 2 == 0:
                            nc.scalar.copy(postg[:, mb * 512:(mb + 1) * 512], pso[:])
                        else:
                            nc.vector.tensor_copy(postg[:, mb * 512:(mb + 1) * 512],
                                                  pso[:])
                        if last:
                            eng = nc.scalar if mb % 2 == 0 else nc.gpsimd
                            eng.dma_start(
                                po[ti * 128:(ti + 1) * 128,
                                   mb * 512:(mb + 1) * 512],
                                postg[:, mb * 512:(mb + 1) * 512])
                    if not last:
                        nc.sync.dma_start(po[ti * 128:(ti + 1) * 128, :], postg[:])

            v_phase()
            pend = {"fin": None}

            for tb in range(len(TBS)):
                c0, c1 = TBS[tb]
                w5 = c1 - c0
                ts5 = slice(c0, c1)
                qraw = []
                for l in range(HL):
                    psq = pps.tile([128, w5], F32, tag="ps")
                    for kc in range(KC):
                        nc.tensor.matmul(
                            psq[:], wq_sb[:, kc * JC + l * 128: kc * JC + (l + 1) * 128],
                            hs_sb[kc][:, ts5], start=(kc == 0), stop=(kc == KC - 1))
                    qraw.append(rope_pre(psq[:], w5))
                if pend["fin"] is not None:
                    pend["fin"]()
                    pend["fin"] = None
                for l in range(HL):
                    rot = rope_rot(qraw[l], pool=pps, tag="ps")
                    rope_post(qraw[l], rot, qrT[l][:, ts5], c0, c1)
                if tb > 0:
                    o_proj(tb - 1)

                cam = []  # per-head deferred CaM state
                psavl = []
                recipl = []
                for l in range(HL):
                    E = []
                    for b in range(NB):
                        pst = pps.tile([128, w5], F32, tag="ps")
                        nc.tensor.matmul(pst[:], krT[l][:, b * 128:(b + 1) * 128],
                                         qrT[l][:, ts5], start=True, stop=True)
                        e = pe16.tile([128, w5], F16, tag="e")
                        nc.scalar.activation(e[:], pst[:], AF.Exp)
                        if b == NB - 1:
                            # drop the non-key row (perm col 1023) from softmax
                            nc.vector.tensor_scalar_mul(e[:], e[:],
                                                        masksf_sb[:, 0:1])
                        E.append(e)
                    psav = pav.tile([128, w5], F32, tag="av")
                    tail_it = (tb == len(TBS) - 1 and l == HL - 1)
                    if tail_it:
                        # denominator first: its recip chain overlaps the AV MMs
                        psdn = ppsd.tile([1, w5], F32, tag="dn")
                        for b in range(NB):
                            nc.tensor.matmul(psdn[:], ones[:], E[b][:],
                                             start=(b == 0), stop=(b == NB - 1))
                        dnf = None
                        dn_sb = prow.tile([1, w5], F32, tag="row512")
                        nc.vector.tensor_copy(dn_sb[:], psdn[:])
                        recip = prow.tile([1, w5], F32, tag="row512")
                        nc.vector.reciprocal(recip[:], dn_sb[:])
                        rbf = ptmp.tile([128, w5], F32, tag="tmp")
                        nc.gpsimd.partition_broadcast(rbf[:], recip[:])
                        for b in range(NB):
                            nc.tensor.matmul(psav[:], vt[b][:, l * D:(l + 1) * D],
                                             E[b][:],
                                             start=(b == 0), stop=(b == NB - 1))
                    else:
                        esum = pe16.tile([128, w5], F16, tag="e")
                        nc.vector.tensor_add(esum[:], E[0][:], E[1][:])
                        for b in range(2, NB):
                            nc.vector.tensor_add(esum[:], esum[:], E[b][:])
                        dnf = ptmp.tile([128, w5], F32, tag="tmp")
                        nc.gpsimd.partition_all_reduce(dnf[:], esum[:], channels=128,
                                                       reduce_op=bass_isa.ReduceOp.add)
                        rbf = ptmp.tile([128, w5], F32, tag="tmp")
                        nc.vector.reciprocal(rbf[:], dnf[:])
                        for b in range(NB):
                            nc.tensor.matmul(psav[:], vt[b][:, l * D:(l + 1) * D],
                                             E[b][:],
                                             start=(b == 0), stop=(b == NB - 1))
                    if tb != CAMTB:
                        nc.vector.tensor_mul(outT[l][:, ts5], psav[:], rbf[:])
                    psavl.append(psav)
                    recipl.append((rbf, dnf))

                    if tb == CAMTB:
                        # ---- CaM scalar chain (PE part deferred past attn l1) ----
                        # sum of E over start+evict rows at CaM cols (255..511)
                        pssA = ppsd.tile([1, 256], F32, tag="dn")
                        nc.tensor.matmul(pssA[:], ones[:], E[0][:, 255:511],
                                         start=True, stop=False)
                        nc.tensor.matmul(pssA[:], mask77A, E[1][:, 255:511],
                                         start=False, stop=True)
                        pssA_sb = prow.tile([1, 256], F32, tag="row256")
                        nc.scalar.copy(pssA_sb[:], pssA[:])
                        # E row of evict key (perm col 204 = block 1 row 76)
                        erow16 = prow.tile([1, 256], F16, tag="row256h")
                        nc.gpsimd.dma_start(erow16[:], E[1][76:77, 255:511])
                        erow = prow.tile([1, 256], F32, tag="row256")
                        nc.vector.tensor_copy(erow[:], erow16[:])
                        srec = prow.tile([1, 256], F32, tag="row256")
                        nc.vector.tensor_sub(srec[:], dnf[0:1, 255:511], pssA_sb[:])
                        # scalars at t = 2047 (perm col 1022 = within-slice 255)
                        r_last = rbf[0:1, 510:511]
                        num = prow.tile([1, 1], F32, tag="sc")
                        nc.vector.tensor_mul(num[:], erow[0:1, 255:256], r_last)
                        mean = prow.tile([1, 1], F32, tag="sc")
                        nc.vector.tensor_mul(mean[:], srec[0:1, 255:256], r_last)
                        nc.vector.tensor_scalar_mul(mean[:], mean[:], 1.0 / 818.0)
                        nc.vector.tensor_scalar_add(mean[:], mean[:], 1e-6)
                        um = prow.tile([1, 1], F32, tag="sc")
                        nc.vector.tensor_mul(um[:], u2_sb[0:1, l:l + 1], mean[:])
                        bern = prow.tile([1, 1], F32, tag="sc")
                        nc.vector.tensor_tensor(bern[:], um[:], num[:],
                                                mybir.AluOpType.is_lt)
                        bs = prow.tile([1, 1], F32, tag="sc")
                        nc.vector.tensor_scalar_mul(bs[:], bern[:], 1.0 / RB)
                        coef_f = prow.tile([1, 256], F32, tag="row256")
                        nc.vector.tensor_scalar_mul(coef_f[:], srec[:], bs[:])
                        coef = prow.tile([1, 256], F16, tag="row256h")
                        nc.vector.tensor_copy(coef[:], coef_f[:])
                        vrow = prow.tile([1, D], F16, tag="vrow")
                        nc.gpsimd.dma_start(vrow[:], vt[1][76:77, l * D:(l + 1) * D])
                        cam.append((coef, vrow))

                # normalize; at tb==CAMTB defer (incl. CaM rank-1) into next tb's
                # slot so the CaM scalar chain never head-blocks the PE queue
                if tb == CAMTB:
                    def fin(ts5=ts5, cam=cam, psavl=psavl, recipl=recipl):
                        for l in range(HL):
                            psav, (rbf, _dnf) = psavl[l], recipl[l]
                            coef, vrow = cam[l]
                            pscr = pps.tile([128, 256], F32, tag="ps")
                            nc.tensor.matmul(pscr[:], vrow[:], coef[:],
                                             start=True, stop=True)
                            nout = ptmp.tile([128, 512], F32, tag="tmp")
                            nc.vector.tensor_mul(nout[:], psav[:], rbf[:])
                            corr = ptmp.tile([128, 256], F32, tag="tmp")
                            nc.vector.tensor_mul(corr[:], pscr[:], rbf[:, 255:511])
                            nc.vector.tensor_add(nout[:, 255:511],
                                                 nout[:, 255:511], corr[:])
                            nc.vector.tensor_copy(outT[l][:, ts5], nout[:])
                    pend["fin"] = fin


            o_proj(len(TBS) - 1)

    nc.compile()
    return nc


_NC_CACHE = None


def _get_nc():
    global _NC_CACHE
    if _NC_CACHE is None:
        _NC_CACHE = _build_nc()
    return _NC_CACHE


PERM = np.concatenate([np.arange(0, SB), np.arange(T - RB, T),
                       np.arange(SB, T - RB)])


def make_in_maps(hidden_states, Wq, Wk, Wv, Wo):
    hs = np.asarray(hidden_states, np.float32).reshape(T, DM)
    hs = np.nan_to_num(hs, nan=0.0, posinf=1e4, neginf=-1e4)
    hsT = np.ascontiguousarray(hs.T[:, PERM].astype(np.float16))
    Wq = np.asarray(Wq, np.float32)
    Wk = np.asarray(Wk, np.float32)
    Wv = np.asarray(Wv, np.float32)
    Wo = np.asarray(Wo, np.float32)

    inv_freq = 1.0 / (10000.0 ** (np.arange(0, D, 2, dtype=np.float32) / D))
    freqs = np.arange(T, dtype=np.float32)[:, None] * inv_freq[None, :]
    emb = np.concatenate([freqs, freqs], axis=-1)          # [T, D]
    cosT = np.ascontiguousarray(np.cos(emb).T[:, PERM].astype(np.float32))
    sinTs = np.ascontiguousarray(np.sin(emb).T[:, PERM].astype(np.float32))
    # rotate-half as a PE stationary: rot(x)[i] = -x[i+64] (i<64), x[i-64] (else)
    prot = np.zeros((D, D), np.float32)
    for i in range(64):
        prot[i + 64, i] = -1.0
        prot[i, i + 64] = 1.0

    import jax
    import jax.numpy as jnp
    u_full = np.asarray(
        jax.random.uniform(jax.random.key(42), (1, H), jnp.float32))

    maskh = np.zeros((128, 2), np.float16)
    maskh[:127, 0] = 1.0      # maskB16: drop row 127 of key block 7
    maskh[:77, 1] = 1.0       # mask77A: start+evict rows of key block 1
    maskf = np.zeros((128, 1), np.float32)
    maskf[:127, 0] = 1.0

    scale = 1.0 / np.sqrt(np.float32(D))

    def wlayout(wT):
        # wT: [DM, JC] -> SBUF layout [128, KC*JC]: [p, kc*JC + j]
        return np.ascontiguousarray(
            wT.reshape(KC, 128, JC).transpose(1, 0, 2).reshape(128, KC * JC)
            .astype(np.float16))

    in_maps = []
    for c in range(NCORES):
        js = slice(c * JC, (c + 1) * JC)
        woT = Wo[:, js].T                                  # [JC, DM]
        wo_l = np.ascontiguousarray(
            woT.reshape(HL, 128, DM).transpose(1, 0, 2).reshape(128, HL * DM)
            .astype(np.float16))
        in_maps.append({
            "hsT": hsT,
            "wq": wlayout(Wq[js, :].T * scale),
            "wk": wlayout(Wk[js, :].T),
            "wv": wlayout(Wv[js, :].T),
            "wo": wo_l,
            "cosT": cosT,
            "sinTs": sinTs,
            "protT": prot,
            "u2": np.ascontiguousarray(u_full[:, c * HL:(c + 1) * HL]),
            "masksh": maskh,
            "masksf": maskf,
        })
    return in_maps


def kernel(hidden_states, Wq, Wk, Wv, Wo):
    nc = _get_nc()
    in_maps = make_in_maps(hidden_states, Wq, Wk, Wv, Wo)
    res = bass_utils.run_bass_kernel_spmd(nc, in_maps,
                                          core_ids=list(range(NCORES)))
    acc = np.zeros((T, DM), np.float32)
    for c in range(NCORES):
        acc += res.results[c]["po"].astype(np.float32)
    out = np.empty((T, DM), np.float32)
    out[PERM] = acc                                       # undo token permutation
    out = np.nan_to_num(out, nan=0.0, posinf=1e4, neginf=-1e4)
    return out.reshape(1, T, DM)


# revision 33
# speedup vs baseline: 1.1656x; 1.0184x over previous
"""Trainium2 Bass kernel for nn_LlamaAttention_cam (sparse attention + CaM merge).

Sharding: tensor-parallel over heads across 8 NeuronCores (2 heads/core).
Each core computes its heads' QKV projections, RoPE, masked attention
(start+recent keep mask), CaM rank-1 correction for the last chunk, and a
partial o_proj.  The host sums the 8 partial outputs (the reduction of the
head-parallel o_proj), replacing the all-reduce.

Token positions are permuted host-side to [start | recent | rest] so the
1023 kept keys occupy the first 1023 columns: key blocks are 8 full
128-blocks (block 7 has one non-key token, masked via an indicator
stationary + a zeroed V row).  The CaM chunk (t in [1792,2048)) lands in
t-block 1 at columns 767..1022, so the serial CaM chain overlaps the rest
of the pipeline instead of sitting in the tail.  hs streams in two waves
(kept tokens first) so K/V projections start early.  o_proj is software-
pipelined one t-block behind attention.  All matmul IO is fp16; PSUM
accumulation fp32; softmax/CaM scalar math fp32.
"""

import sys

for _p in ("/opt/trn_rl_repo",):
    if _p not in sys.path:
        sys.path.append(_p)

import numpy as np

import concourse.bass as bass
import concourse.bass_isa as bass_isa
import concourse.mybir as mybir
import concourse.tile as tile
from concourse import bacc, bass_utils

F32 = mybir.dt.float32
F16 = mybir.dt.float16
AF = mybir.ActivationFunctionType

T = 2048
DM = 2048
H = 16
D = 128
NCORES = 8
HL = H // NCORES          # heads per core = 2
JC = HL * D               # local attn width = 256
SB = 204                  # start keep
RB = 819                  # recent keep
KC = DM // 128            # 16 model-dim chunks
TB = T // 512             # 4 t-blocks of 512
TBS = [(0, 512), (512, 1024), (1024, 1536), (1536, 2048)]
NB = 8                    # kept-key blocks (perm cols 0..1024, col 1023 masked)
# perm order: [0..204) + [1229..2048) + [204..1229)
# CaM: evict key 1229 -> perm col 204 (block 1, row 76)
# CaM q range t in [1792,2048) -> perm cols 767..1022 = tb1 local cols 255..510
CAMTB = 1


def _build_nc():
    nc = bacc.Bacc("TRN2", target_bir_lowering=False, debug=False,
                   num_devices=NCORES)
    hsT = nc.dram_tensor("hsT", [DM, T], F16, kind="ExternalInput").ap()
    wq = nc.dram_tensor("wq", [128, KC * JC], F16, kind="ExternalInput").ap()
    wk = nc.dram_tensor("wk", [128, KC * JC], F16, kind="ExternalInput").ap()
    wv = nc.dram_tensor("wv", [128, KC * JC], F16, kind="ExternalInput").ap()
    wo = nc.dram_tensor("wo", [128, HL * DM], F16, kind="ExternalInput").ap()
    cosT = nc.dram_tensor("cosT", [D, T], F32, kind="ExternalInput").ap()
    sinTs = nc.dram_tensor("sinTs", [D, T], F32, kind="ExternalInput").ap()
    protT = nc.dram_tensor("protT", [D, D],
                           mybir.dt.float32r, kind="ExternalInput").ap()
    u2 = nc.dram_tensor("u2", [1, HL], F32, kind="ExternalInput").ap()
    masksh = nc.dram_tensor("masksh", [128, 2], F16, kind="ExternalInput").ap()
    masksf = nc.dram_tensor("masksf", [128, 1], F32, kind="ExternalInput").ap()
    po = nc.dram_tensor("po", [T, DM], F16, kind="ExternalOutput").ap()

    with tile.TileContext(nc) as tc:
        with (
            tc.tile_pool(name="resid", bufs=1) as pres,        # long-lived
            tc.tile_pool(name="e16", bufs=12) as pe16,         # f16 [128,512] E tiles
            tc.tile_pool(name="tmp", bufs=8) as ptmp,          # f32 [128,512] transients
            tc.tile_pool(name="post", bufs=3) as ppost,        # f16 [128,2048] out staging
            tc.tile_pool(name="rows", bufs=4) as prow,         # small [1,*] tiles
            tc.tile_pool(name="ps", bufs=4, space="PSUM") as pps,
            tc.tile_pool(name="psav", bufs=2, space="PSUM") as pav,
            tc.tile_pool(name="psdn", bufs=2, space="PSUM") as ppsd,
        ):
            # ---- input DMAs: wk first, then kept-token hs wave, then the rest ----
            wk_sb = pres.tile([128, KC * JC], F16, tag="wk")
            hs_sb = [pres.tile([128, T], F16, tag=f"hs{kc}", name=f"hs{kc}")
                     for kc in range(KC)]
            wv_sb = pres.tile([128, KC * JC], F16, tag="wv")
            nc.sync.dma_start(wk_sb[:, 0:512], wk[:, 0:512])
            nc.sync.dma_start(hs_sb[0][:, 0:1024], hsT[0:128, 0:1024])
            nc.sync.dma_start(hs_sb[1][:, 0:1024], hsT[128:256, 0:1024])
            nc.sync.dma_start(wk_sb[:, 512:1024], wk[:, 512:1024])
            for kc in range(2, 4):
                nc.sync.dma_start(hs_sb[kc][:, 0:1024],
                                  hsT[kc * 128:(kc + 1) * 128, 0:1024])
            nc.sync.dma_start(wk_sb[:, 1024:2048], wk[:, 1024:2048])
            for kc in range(4, 8):
                nc.sync.dma_start(hs_sb[kc][:, 0:1024],
                                  hsT[kc * 128:(kc + 1) * 128, 0:1024])
            nc.sync.dma_start(wk_sb[:, 2048:4096], wk[:, 2048:4096])
            for kc in range(8, KC):
                nc.sync.dma_start(hs_sb[kc][:, 0:1024],
                                  hsT[kc * 128:(kc + 1) * 128, 0:1024])
            for qq in range(4):
                nc.sync.dma_start(wv_sb[:, qq * 1024:(qq + 1) * 1024],
                                  wv[:, qq * 1024:(qq + 1) * 1024])
            cosT_sb = pres.tile([D, T], F32, tag="cos")
            sinTs_sb = pres.tile([D, T], F32, tag="sin")
            protT_sb = pres.tile([D, D], mybir.dt.float32r, tag="prot")
            nc.sync.dma_start(cosT_sb[:], cosT[:])
            nc.sync.dma_start(sinTs_sb[:], sinTs[:])
            nc.sync.dma_start(protT_sb[:], protT[:])
            wq_sb = pres.tile([128, KC * JC], F16, tag="wq")
            nc.sync.dma_start(wq_sb[:], wq[:])
            for kc in range(KC):
                nc.sync.dma_start(hs_sb[kc][:, 1024:2048],
                                  hsT[kc * 128:(kc + 1) * 128, 1024:2048])
            wo_sb = pres.tile([128, HL * DM], F16, tag="wo")
            nc.sync.dma_start(wo_sb[:], wo[:])
            u2_sb = pres.tile([1, HL], F32, tag="u2")
            nc.sync.dma_start(u2_sb[:], u2[:])
            masksh_sb = pres.tile([128, 2], F16, tag="masksh")
            nc.sync.dma_start(masksh_sb[:], masksh[:])
            masksf_sb = pres.tile([128, 1], F32, tag="masksf")
            nc.sync.dma_start(masksf_sb[:], masksf[:])
            maskB16 = masksh_sb[:, 0:1]    # rows <= 126 (drop key col 1023)
            mask77A = masksh_sb[:, 1:2]    # rows <= 76 (start + evict rows of blk 1)

            ones_f = pres.tile([128, 1], F32, tag="ones_f")
            nc.vector.memset(ones_f[:], 1.0)
            ones = pres.tile([128, 1], F16, tag="ones")
            nc.vector.tensor_copy(ones[:], ones_f[:])

            # rope'd q/k in [d, t] layout; v in [t, d_local] layout
            qrT = [pres.tile([D, T], F16, tag=f"qrT{l}", name=f"qrT{l}") for l in range(HL)]
            krT = [pres.tile([D, T], F16, tag=f"krT{l}", name=f"krT{l}") for l in range(HL)]
            vt = [pres.tile([128, JC], F16, tag=f"vt{b}", name=f"vt{b}") for b in range(NB)]
            outT = [pres.tile([D, T], F16, tag=f"outT{l}", name=f"outT{l}") for l in range(HL)]

            F32R = mybir.dt.float32r

            def rope_pre(ps_ap, w):
                raw = ptmp.tile([128, w], F32R, tag="tmp")
                nc.scalar.copy(raw[:], ps_ap)
                return raw

            def rope_rot(raw, pool=None, tag="av"):
                rot = (pool or pav).tile([128, raw.shape[1]], F32, tag=tag)
                nc.tensor.matmul(rot[:], protT_sb[:], raw[:],
                                 start=True, stop=True)
                return rot

            def rope_post(raw, rot, dst_ap, c0, c1):
                w = c1 - c0
                t1 = ptmp.tile([128, w], F32, tag="tmp")
                nc.gpsimd.tensor_mul(t1[:], raw[:].bitcast(F32), cosT_sb[:, c0:c1])
                t2 = ptmp.tile([128, w], F32, tag="tmp")
                nc.vector.tensor_mul(t2[:], rot[:], sinTs_sb[:, c0:c1])
                nc.vector.tensor_add(dst_ap, t1[:], t2[:])

            # ---------------- phase K: kept keys (perm cols 0..1024), kc-outer ----
            psk = []
            for l in range(HL):
                pska = pps.tile([128, 512], F32, tag="ps", name=f"pska{l}")
                pskb = pps.tile([128, 512], F32, tag="ps", name=f"pskb{l}")
                psk.append((pska, pskb))
            for kc in range(KC):
                for l in range(HL):
                    wsl = wk_sb[:, kc * JC + l * 128: kc * JC + (l + 1) * 128]
                    nc.tensor.matmul(psk[l][0][:], wsl, hs_sb[kc][:, 0:512],
                                     start=(kc == 0), stop=(kc == KC - 1))
                    nc.tensor.matmul(psk[l][1][:], wsl, hs_sb[kc][:, 512:1024],
                                     start=(kc == 0), stop=(kc == KC - 1))
            kraw = []
            for l in range(HL):
                kraw.append((rope_pre(psk[l][0][:], 512),
                             rope_pre(psk[l][1][:], 512)))

            # ---------------- phase V (paired blocks, issued after q-tb0) ----------
            def v_phase():
                for bp in range(0, NB, 2):
                    if bp == 2:
                        for l in range(HL):
                            ra, rb = kraw[l]
                            rota = rope_rot(ra)
                            rope_post(ra, rota, krT[l][:, 0:512], 0, 512)
                            rotb = rope_rot(rb)
                            rope_post(rb, rotb, krT[l][:, 512:1024], 512, 1024)
                    psvA = pps.tile([128, JC], F32, tag="ps", name=f"psvA{bp}")
                    psvB = pps.tile([128, JC], F32, tag="ps", name=f"psvB{bp}")
                    for kc in range(KC):
                        nc.tensor.matmul(psvA[:], hs_sb[kc][:, bp * 128:(bp + 1) * 128],
                                         wv_sb[:, kc * JC:(kc + 1) * JC],
                                         start=(kc == 0), stop=(kc == KC - 1))
                        nc.tensor.matmul(psvB[:],
                                         hs_sb[kc][:, (bp + 1) * 128:(bp + 2) * 128],
                                         wv_sb[:, kc * JC:(kc + 1) * JC],
                                         start=(kc == 0), stop=(kc == KC - 1))
                    nc.scalar.copy(vt[bp][:], psvA[:])
                    nc.vector.tensor_copy(vt[bp + 1][:], psvB[:])
                    if bp + 1 == NB - 1:
                        # zero the non-key row (perm col 1023) so AV ignores it
                        nc.vector.tensor_scalar_mul(vt[bp + 1][:], vt[bp + 1][:],
                                                    masksf_sb[:, 0:1])

            # ---------------- phase Q + attention + pipelined o_proj ----------------
            def o_proj(tb):
                c0, c1 = TBS[tb]
                for ti in range(c0 // 128, c1 // 128):
                    last = (ti == 4 * TB - 1)
                    postg = ppost.tile([128, DM], F16, tag="post")
                    for mb in range(TB):
                        pso = pps.tile([128, 512], F32, tag="ps")
                        for l in range(HL):
                            nc.tensor.matmul(
                                pso[:], outT[l][:, ti * 128:(ti + 1) * 128],
                                wo_sb[:, l * DM + mb * 512: l * DM + (mb + 1) * 512],
                                start=(l == 0), stop=(l == HL - 1))
                        if mb % 2 == 0:
                            nc.scalar.copy(postg[:, mb * 512:(mb + 1) * 512], pso[:])
                        else:
                            nc.vector.tensor_copy(postg[:, mb * 512:(mb + 1) * 512],
                                                  pso[:])
                        if last:
                            eng = nc.scalar if mb % 2 == 0 else nc.gpsimd
                            eng.dma_start(
                                po[ti * 128:(ti + 1) * 128,
                                   mb * 512:(mb + 1) * 512],
                                postg[:, mb * 512:(mb + 1) * 512])
                    if not last:
                        nc.sync.dma_start(po[ti * 128:(ti + 1) * 128, :], postg[:])

            v_phase()
            pend = {"fin": None}

            for tb in range(len(TBS)):
                c0, c1 = TBS[tb]
                w5 = c1 - c0
                ts5 = slice(c0, c1)
                qraw = []
                for l in range(HL):
                    psq = pps.tile([128, w5], F32, tag="ps")
                    for kc in range(KC):
                        nc.tensor.matmul(
                            psq[:], wq_sb[:, kc * JC + l * 128: kc * JC + (l + 1) * 128],
                            hs_sb[kc][:, ts5], start=(kc == 0), stop=(kc == KC - 1))
                    qraw.append(rope_pre(psq[:], w5))
                if pend["fin"] is not None:
                    pend["fin"]()
                    pend["fin"] = None
                for l in range(HL):
                    rot = rope_rot(qraw[l], pool=pps, tag="ps")
                    rope_post(qraw[l], rot, qrT[l][:, ts5], c0, c1)
                if tb > 0:
                    o_proj(tb - 1)

                cam = []  # per-head deferred CaM state
                psavl = []
                recipl = []
                for l in range(HL):
                    E = []
                    for b in range(NB):
                        pst = pps.tile([128, w5], F32, tag="ps")
                        nc.tensor.matmul(pst[:], krT[l][:, b * 128:(b + 1) * 128],
                                         qrT[l][:, ts5], start=True, stop=True)
                        e = pe16.tile([128, w5], F16, tag="e")
                        nc.scalar.activation(e[:], pst[:], AF.Exp)
                        if b == NB - 1:
                            # drop the non-key row (perm col 1023) from softmax
                            nc.vector.tensor_scalar_mul(e[:], e[:],
                                                        masksf_sb[:, 0:1])
                        E.append(e)
                    psav = pav.tile([128, w5], F32, tag="av")
                    tail_it = (tb == len(TBS) - 1 and l == HL - 1)
                    if tail_it:
                        # denominator first: its recip chain overlaps the AV MMs
                        psdn = ppsd.tile([1, w5], F32, tag="dn")
                        for b in range(NB):
                            nc.tensor.matmul(psdn[:], ones[:], E[b][:],
                                             start=(b == 0), stop=(b == NB - 1))
                        dnf = None
                        recip = prow.tile([1, w5], F32, tag="row512")
                        nc.vector.reciprocal(recip[:], psdn[0:1, :])
                        rbf = ptmp.tile([128, w5], F32, tag="tmp")
                        nc.gpsimd.partition_broadcast(rbf[:], recip[:])
                        for b in range(NB):
                            nc.tensor.matmul(psav[:], vt[b][:, l * D:(l + 1) * D],
                                             E[b][:],
                                             start=(b == 0), stop=(b == NB - 1))
                    else:
                        esum = pe16.tile([128, w5], F16, tag="e")
                        nc.vector.tensor_add(esum[:], E[0][:], E[1][:])
                        for b in range(2, NB):
                            nc.vector.tensor_add(esum[:], esum[:], E[b][:])
                        dnf = ptmp.tile([128, w5], F32, tag="tmp")
                        nc.gpsimd.partition_all_reduce(dnf[:], esum[:], channels=128,
                                                       reduce_op=bass_isa.ReduceOp.add)
                        rbf = ptmp.tile([128, w5], F32, tag="tmp")
                        nc.vector.reciprocal(rbf[:], dnf[:])
                        for b in range(NB):
                            nc.tensor.matmul(psav[:], vt[b][:, l * D:(l + 1) * D],
                                             E[b][:],
                                             start=(b == 0), stop=(b == NB - 1))
                    if tb != CAMTB:
                        nc.vector.tensor_mul(outT[l][:, ts5], psav[:], rbf[:])
                    psavl.append(psav)
                    recipl.append((rbf, dnf))

                    if tb == CAMTB:
                        # ---- CaM scalar chain (PE part deferred past attn l1) ----
                        # sum of E over start+evict rows at CaM cols (255..511)
                        pssA = ppsd.tile([1, 256], F32, tag="dn")
                        nc.tensor.matmul(pssA[:], ones[:], E[0][:, 255:511],
                                         start=True, stop=False)
                        nc.tensor.matmul(pssA[:], mask77A, E[1][:, 255:511],
                                         start=False, stop=True)
                        pssA_sb = prow.tile([1, 256], F32, tag="row256")
                        nc.scalar.copy(pssA_sb[:], pssA[:])
                        # E row of evict key (perm col 204 = block 1 row 76)
                        erow16 = prow.tile([1, 256], F16, tag="row256h")
                        nc.gpsimd.dma_start(erow16[:], E[1][76:77, 255:511])
                        erow = prow.tile([1, 256], F32, tag="row256")
                        nc.vector.tensor_copy(erow[:], erow16[:])
                        srec = prow.tile([1, 256], F32, tag="row256")
                        nc.vector.tensor_sub(srec[:], dnf[0:1, 255:511], pssA_sb[:])
                        # scalars at t = 2047 (perm col 1022 = within-slice 255)
                        r_last = rbf[0:1, 510:511]
                        num = prow.tile([1, 1], F32, tag="sc")
                        nc.vector.tensor_mul(num[:], erow[0:1, 255:256], r_last)
                        mean = prow.tile([1, 1], F32, tag="sc")
                        nc.vector.tensor_mul(mean[:], srec[0:1, 255:256], r_last)
                        nc.vector.tensor_scalar_mul(mean[:], mean[:], 1.0 / 818.0)
                        nc.vector.tensor_scalar_add(mean[:], mean[:], 1e-6)
                        um = prow.tile([1, 1], F32, tag="sc")
                        nc.vector.tensor_mul(um[:], u2_sb[0:1, l:l + 1], mean[:])
                        bern = prow.tile([1, 1], F32, tag="sc")
                        nc.vector.tensor_tensor(bern[:], um[:], num[:],
                                                mybir.AluOpType.is_lt)
                        bs = prow.tile([1, 1], F32, tag="sc")
                        nc.vector.tensor_scalar_mul(bs[:], bern[:], 1.0 / RB)
                        coef_f = prow.tile([1, 256], F32, tag="row256")
                        nc.vector.tensor_scalar_mul(coef_f[:], srec[:], bs[:])
                        coef = prow.tile([1, 256], F16, tag="row256h")
                        nc.vector.tensor_copy(coef[:], coef_f[:])
                        vrow = prow.tile([1, D], F16, tag="vrow")
                        nc.gpsimd.dma_start(vrow[:], vt[1][76:77, l * D:(l + 1) * D])
                        cam.append((coef, vrow))

                # normalize; at tb==CAMTB defer (incl. CaM rank-1) into next tb's
                # slot so the CaM scalar chain never head-blocks the PE queue
                if tb == CAMTB:
                    def fin(ts5=ts5, cam=cam, psavl=psavl, recipl=recipl):
                        for l in range(HL):
                            psav, (rbf, _dnf) = psavl[l], recipl[l]
                            coef, vrow = cam[l]
                            pscr = pps.tile([128, 256], F32, tag="ps")
                            nc.tensor.matmul(pscr[:], vrow[:], coef[:],
                                             start=True, stop=True)
                            nout = ptmp.tile([128, 512], F32, tag="tmp")
                            nc.vector.tensor_mul(nout[:], psav[:], rbf[:])
                            corr = ptmp.tile([128, 256], F32, tag="tmp")
                            nc.vector.tensor_mul(corr[:], pscr[:], rbf[:, 255:511])
                            nc.vector.tensor_add(nout[:, 255:511],
                                                 nout[:, 255:511], corr[:])
                            nc.vector.tensor_copy(outT[l][:, ts5], nout[:])
                    pend["fin"] = fin


            o_proj(len(TBS) - 1)

    nc.compile()
    return nc


_NC_CACHE = None


def _get_nc():
    global _NC_CACHE
    if _NC_CACHE is None:
        _NC_CACHE = _build_nc()
    return _NC_CACHE


PERM = np.concatenate([np.arange(0, SB), np.arange(T - RB, T),
                       np.arange(SB, T - RB)])


def make_in_maps(hidden_states, Wq, Wk, Wv, Wo):
    hs = np.asarray(hidden_states, np.float32).reshape(T, DM)
    hs = np.nan_to_num(hs, nan=0.0, posinf=1e4, neginf=-1e4)
    hsT = np.ascontiguousarray(hs.T[:, PERM].astype(np.float16))
    Wq = np.asarray(Wq, np.float32)
    Wk = np.asarray(Wk, np.float32)
    Wv = np.asarray(Wv, np.float32)
    Wo = np.asarray(Wo, np.float32)

    inv_freq = 1.0 / (10000.0 ** (np.arange(0, D, 2, dtype=np.float32) / D))
    freqs = np.arange(T, dtype=np.float32)[:, None] * inv_freq[None, :]
    emb = np.concatenate([freqs, freqs], axis=-1)          # [T, D]
    cosT = np.ascontiguousarray(np.cos(emb).T[:, PERM].astype(np.float32))
    sinTs = np.ascontiguousarray(np.sin(emb).T[:, PERM].astype(np.float32))
    # rotate-half as a PE stationary: rot(x)[i] = -x[i+64] (i<64), x[i-64] (else)
    prot = np.zeros((D, D), np.float32)
    for i in range(64):
        prot[i + 64, i] = -1.0
        prot[i, i + 64] = 1.0

    import jax
    import jax.numpy as jnp
    u_full = np.asarray(
        jax.random.uniform(jax.random.key(42), (1, H), jnp.float32))

    maskh = np.zeros((128, 2), np.float16)
    maskh[:127, 0] = 1.0      # maskB16: drop row 127 of key block 7
    maskh[:77, 1] = 1.0       # mask77A: start+evict rows of key block 1
    maskf = np.zeros((128, 1), np.float32)
    maskf[:127, 0] = 1.0

    scale = 1.0 / np.sqrt(np.float32(D))

    def wlayout(wT):
        # wT: [DM, JC] -> SBUF layout [128, KC*JC]: [p, kc*JC + j]
        return np.ascontiguousarray(
            wT.reshape(KC, 128, JC).transpose(1, 0, 2).reshape(128, KC * JC)
            .astype(np.float16))

    in_maps = []
    for c in range(NCORES):
        js = slice(c * JC, (c + 1) * JC)
        woT = Wo[:, js].T                                  # [JC, DM]
        wo_l = np.ascontiguousarray(
            woT.reshape(HL, 128, DM).transpose(1, 0, 2).reshape(128, HL * DM)
            .astype(np.float16))
        in_maps.append({
            "hsT": hsT,
            "wq": wlayout(Wq[js, :].T * scale),
            "wk": wlayout(Wk[js, :].T),
            "wv": wlayout(Wv[js, :].T),
            "wo": wo_l,
            "cosT": cosT,
            "sinTs": sinTs,
            "protT": prot,
            "u2": np.ascontiguousarray(u_full[:, c * HL:(c + 1) * HL]),
            "masksh": maskh,
            "masksf": maskf,
        })
    return in_maps


def kernel(hidden_states, Wq, Wk, Wv, Wo):
    nc = _get_nc()
    in_maps = make_in_maps(hidden_states, Wq, Wk, Wv, Wo)
    res = bass_utils.run_bass_kernel_spmd(nc, in_maps,
                                          core_ids=list(range(NCORES)))
    acc = np.zeros((T, DM), np.float32)
    for c in range(NCORES):
        acc += res.results[c]["po"].astype(np.float32)
    out = np.empty((T, DM), np.float32)
    out[PERM] = acc                                       # undo token permutation
    out = np.nan_to_num(out, nan=0.0, posinf=1e4, neginf=-1e4)
    return out.reshape(1, T, DM)


# revision 34
# speedup vs baseline: 38927.5703x; 33395.8882x over previous
"""Trainium2 Bass kernel for nn_LlamaAttention_cam (sparse attention + CaM merge).

Sharding: tensor-parallel over heads across 8 NeuronCores (2 heads/core).
Each core computes its heads' QKV projections, RoPE, masked attention
(start+recent keep mask), CaM rank-1 correction for the last chunk, and a
partial o_proj.  The host sums the 8 partial outputs (the reduction of the
head-parallel o_proj), replacing the all-reduce.

Token positions are permuted host-side to [start | recent | rest] so the
1023 kept keys occupy the first 1023 columns: key blocks are 8 full
128-blocks (block 7 has one non-key token, masked via an indicator
stationary + a zeroed V row).  The CaM chunk (t in [1792,2048)) lands in
t-block 1 at columns 767..1022, so the serial CaM chain overlaps the rest
of the pipeline instead of sitting in the tail.  hs streams in two waves
(kept tokens first) so K/V projections start early.  o_proj is software-
pipelined one t-block behind attention.  All matmul IO is fp16; PSUM
accumulation fp32; softmax/CaM scalar math fp32.
"""

import sys

for _p in ("/opt/trn_rl_repo",):
    if _p not in sys.path:
        sys.path.append(_p)

import numpy as np

import concourse.bass as bass
import concourse.bass_isa as bass_isa
import concourse.mybir as mybir
import concourse.tile as tile
from concourse import bacc, bass_utils

F32 = mybir.dt.float32
F16 = mybir.dt.float16
AF = mybir.ActivationFunctionType

T = 2048
DM = 2048
H = 16
D = 128
NCORES = 8
HL = H // NCORES          # heads per core = 2
JC = HL * D               # local attn width = 256
SB = 204                  # start keep
RB = 819                  # recent keep
KC = DM // 128            # 16 model-dim chunks
TB = T // 512             # 4 t-blocks of 512
TBS = [(0, 512), (512, 1024), (1024, 1536), (1536, 2048)]
NB = 8                    # kept-key blocks (perm cols 0..1024, col 1023 masked)
# perm order: [0..204) + [1229..2048) + [204..1229)
# CaM: evict key 1229 -> perm col 204 (block 1, row 76)
# CaM q range t in [1792,2048) -> perm cols 767..1022 = tb1 local cols 255..510
CAMTB = 1


def _build_nc():
    nc = bacc.Bacc("TRN2", target_bir_lowering=False, debug=False,
                   num_devices=NCORES)
    hsT = nc.dram_tensor("hsT", [DM, T], F16, kind="ExternalInput").ap()
    wq = nc.dram_tensor("wq", [128, KC * JC], F16, kind="ExternalInput").ap()
    wk = nc.dram_tensor("wk", [128, KC * JC], F16, kind="ExternalInput").ap()
    wv = nc.dram_tensor("wv", [128, KC * JC], F16, kind="ExternalInput").ap()
    wo = nc.dram_tensor("wo", [128, HL * DM], F16, kind="ExternalInput").ap()
    cosT = nc.dram_tensor("cosT", [D, T], F32, kind="ExternalInput").ap()
    sinTs = nc.dram_tensor("sinTs", [D, T], F32, kind="ExternalInput").ap()
    protT = nc.dram_tensor("protT", [D, D],
                           mybir.dt.float32r, kind="ExternalInput").ap()
    u2 = nc.dram_tensor("u2", [1, HL], F32, kind="ExternalInput").ap()
    masksh = nc.dram_tensor("masksh", [128, 2], F16, kind="ExternalInput").ap()
    masksf = nc.dram_tensor("masksf", [128, 1], F32, kind="ExternalInput").ap()
    po = nc.dram_tensor("po", [T, DM], F16, kind="ExternalOutput").ap()

    with tile.TileContext(nc) as tc:
        with (
            tc.tile_pool(name="resid", bufs=1) as pres,        # long-lived
            tc.tile_pool(name="e16", bufs=12) as pe16,         # f16 [128,512] E tiles
            tc.tile_pool(name="tmp", bufs=8) as ptmp,          # f32 [128,512] transients
            tc.tile_pool(name="post", bufs=3) as ppost,        # f16 [128,2048] out staging
            tc.tile_pool(name="rows", bufs=4) as prow,         # small [1,*] tiles
            tc.tile_pool(name="ps", bufs=4, space="PSUM") as pps,
            tc.tile_pool(name="psav", bufs=2, space="PSUM") as pav,
            tc.tile_pool(name="psdn", bufs=2, space="PSUM") as ppsd,
        ):
            # ---- input DMAs: wk first, then kept-token hs wave, then the rest ----
            wk_sb = pres.tile([128, KC * JC], F16, tag="wk")
            hs_sb = [pres.tile([128, T], F16, tag=f"hs{kc}", name=f"hs{kc}")
                     for kc in range(KC)]
            wv_sb = pres.tile([128, KC * JC], F16, tag="wv")
            nc.sync.dma_start(wk_sb[:, 0:512], wk[:, 0:512])
            nc.sync.dma_start(hs_sb[0][:, 0:512], hsT[0:128, 0:512])
            nc.sync.dma_start(hs_sb[0][:, 512:1024], hsT[0:128, 512:1024])
            nc.sync.dma_start(hs_sb[1][:, 0:1024], hsT[128:256, 0:1024])
            nc.sync.dma_start(wk_sb[:, 512:1024], wk[:, 512:1024])
            for kc in range(2, 4):
                nc.sync.dma_start(hs_sb[kc][:, 0:1024],
                                  hsT[kc * 128:(kc + 1) * 128, 0:1024])
            nc.sync.dma_start(wk_sb[:, 1024:2048], wk[:, 1024:2048])
            for kc in range(4, 8):
                nc.sync.dma_start(hs_sb[kc][:, 0:1024],
                                  hsT[kc * 128:(kc + 1) * 128, 0:1024])
            nc.sync.dma_start(wk_sb[:, 2048:4096], wk[:, 2048:4096])
            for kc in range(8, KC):
                nc.sync.dma_start(hs_sb[kc][:, 0:1024],
                                  hsT[kc * 128:(kc + 1) * 128, 0:1024])
            for qq in range(4):
                nc.sync.dma_start(wv_sb[:, qq * 1024:(qq + 1) * 1024],
                                  wv[:, qq * 1024:(qq + 1) * 1024])
            cosT_sb = pres.tile([D, T], F32, tag="cos")
            sinTs_sb = pres.tile([D, T], F32, tag="sin")
            protT_sb = pres.tile([D, D], mybir.dt.float32r, tag="prot")
            nc.sync.dma_start(cosT_sb[:], cosT[:])
            nc.sync.dma_start(sinTs_sb[:], sinTs[:])
            nc.sync.dma_start(protT_sb[:], protT[:])
            wq_sb = pres.tile([128, KC * JC], F16, tag="wq")
            nc.sync.dma_start(wq_sb[:], wq[:])
            for kc in range(KC):
                nc.sync.dma_start(hs_sb[kc][:, 1024:2048],
                                  hsT[kc * 128:(kc + 1) * 128, 1024:2048])
            wo_sb = pres.tile([128, HL * DM], F16, tag="wo")
            nc.sync.dma_start(wo_sb[:], wo[:])
            u2_sb = pres.tile([1, HL], F32, tag="u2")
            nc.sync.dma_start(u2_sb[:], u2[:])
            masksh_sb = pres.tile([128, 2], F16, tag="masksh")
            nc.sync.dma_start(masksh_sb[:], masksh[:])
            masksf_sb = pres.tile([128, 1], F32, tag="masksf")
            nc.sync.dma_start(masksf_sb[:], masksf[:])
            maskB16 = masksh_sb[:, 0:1]    # rows <= 126 (drop key col 1023)
            mask77A = masksh_sb[:, 1:2]    # rows <= 76 (start + evict rows of blk 1)

            ones_f = pres.tile([128, 1], F32, tag="ones_f")
            nc.vector.memset(ones_f[:], 1.0)
            ones = pres.tile([128, 1], F16, tag="ones")
            nc.vector.tensor_copy(ones[:], ones_f[:])

            # rope'd q/k in [d, t] layout; v in [t, d_local] layout
            qrT = [pres.tile([D, T], F16, tag=f"qrT{l}", name=f"qrT{l}") for l in range(HL)]
            krT = [pres.tile([D, T], F16, tag=f"krT{l}", name=f"krT{l}") for l in range(HL)]
            vt = [pres.tile([128, JC], F16, tag=f"vt{b}", name=f"vt{b}") for b in range(NB)]
            outT = [pres.tile([D, T], F16, tag=f"outT{l}", name=f"outT{l}") for l in range(HL)]

            F32R = mybir.dt.float32r

            def rope_pre(ps_ap, w):
                raw = ptmp.tile([128, w], F32R, tag="tmp")
                nc.scalar.copy(raw[:], ps_ap)
                return raw

            def rope_rot(raw, pool=None, tag="av"):
                rot = (pool or pav).tile([128, raw.shape[1]], F32, tag=tag)
                nc.tensor.matmul(rot[:], protT_sb[:], raw[:],
                                 start=True, stop=True)
                return rot

            def rope_post(raw, rot, dst_ap, c0, c1):
                w = c1 - c0
                t1 = ptmp.tile([128, w], F32, tag="tmp")
                nc.gpsimd.tensor_mul(t1[:], raw[:].bitcast(F32), cosT_sb[:, c0:c1])
                t2 = ptmp.tile([128, w], F32, tag="tmp")
                nc.vector.tensor_mul(t2[:], rot[:], sinTs_sb[:, c0:c1])
                nc.vector.tensor_add(dst_ap, t1[:], t2[:])

            # ---------------- phase K: kept keys (perm cols 0..1024), kc-outer ----
            psk = []
            for l in range(HL):
                pska = pps.tile([128, 512], F32, tag="ps", name=f"pska{l}")
                pskb = pps.tile([128, 512], F32, tag="ps", name=f"pskb{l}")
                psk.append((pska, pskb))
            for kc in range(KC):
                for l in range(HL):
                    wsl = wk_sb[:, kc * JC + l * 128: kc * JC + (l + 1) * 128]
                    nc.tensor.matmul(psk[l][0][:], wsl, hs_sb[kc][:, 0:512],
                                     start=(kc == 0), stop=(kc == KC - 1))
                    nc.tensor.matmul(psk[l][1][:], wsl, hs_sb[kc][:, 512:1024],
                                     start=(kc == 0), stop=(kc == KC - 1))
            kraw = []
            for l in range(HL):
                kraw.append((rope_pre(psk[l][0][:], 512),
                             rope_pre(psk[l][1][:], 512)))

            # ---------------- phase V (paired blocks, issued after q-tb0) ----------
            def v_phase():
                for bp in range(0, NB, 2):
                    if bp == 2:
                        for l in range(HL):
                            ra, rb = kraw[l]
                            rota = rope_rot(ra)
                            rope_post(ra, rota, krT[l][:, 0:512], 0, 512)
                            rotb = rope_rot(rb)
                            rope_post(rb, rotb, krT[l][:, 512:1024], 512, 1024)
                    psvA = pps.tile([128, JC], F32, tag="ps", name=f"psvA{bp}")
                    psvB = pps.tile([128, JC], F32, tag="ps", name=f"psvB{bp}")
                    for kc in range(KC):
                        nc.tensor.matmul(psvA[:], hs_sb[kc][:, bp * 128:(bp + 1) * 128],
                                         wv_sb[:, kc * JC:(kc + 1) * JC],
                                         start=(kc == 0), stop=(kc == KC - 1))
                        nc.tensor.matmul(psvB[:],
                                         hs_sb[kc][:, (bp + 1) * 128:(bp + 2) * 128],
                                         wv_sb[:, kc * JC:(kc + 1) * JC],
                                         start=(kc == 0), stop=(kc == KC - 1))
                    nc.scalar.copy(vt[bp][:], psvA[:])
                    nc.vector.tensor_copy(vt[bp + 1][:], psvB[:])
                    if bp + 1 == NB - 1:
                        # zero the non-key row (perm col 1023) so AV ignores it
                        nc.vector.tensor_scalar_mul(vt[bp + 1][:], vt[bp + 1][:],
                                                    masksf_sb[:, 0:1])

            # ---------------- phase Q + attention + pipelined o_proj ----------------
            def o_proj(tb):
                c0, c1 = TBS[tb]
                for ti in range(c0 // 128, c1 // 128):
                    last = (ti == 4 * TB - 1)
                    postg = ppost.tile([128, DM], F16, tag="post")
                    for mb in range(TB):
                        if mb < 2:
                            pso = pps.tile([128, 512], F32, tag="ps")
                        else:
                            pso = pav.tile([128, 512], F32, tag="av")
                        for l in range(HL):
                            nc.tensor.matmul(
                                pso[:], outT[l][:, ti * 128:(ti + 1) * 128],
                                wo_sb[:, l * DM + mb * 512: l * DM + (mb + 1) * 512],
                                start=(l == 0), stop=(l == HL - 1))
                        if mb % 2 == 0:
                            nc.scalar.copy(postg[:, mb * 512:(mb + 1) * 512], pso[:])
                        else:
                            nc.vector.tensor_copy(postg[:, mb * 512:(mb + 1) * 512],
                                                  pso[:])
                        if last:
                            eng = nc.scalar if mb % 2 == 0 else nc.gpsimd
                            eng.dma_start(
                                po[ti * 128:(ti + 1) * 128,
                                   mb * 512:(mb + 1) * 512],
                                postg[:, mb * 512:(mb + 1) * 512])
                    if not last:
                        nc.sync.dma_start(po[ti * 128:(ti + 1) * 128, :], postg[:])

            v_phase()
            pend = {"fin": None}

            for tb in range(len(TBS)):
                c0, c1 = TBS[tb]
                w5 = c1 - c0
                ts5 = slice(c0, c1)
                qraw = []
                for l in range(HL):
                    psq = pps.tile([128, w5], F32, tag="ps")
                    for kc in range(KC):
                        nc.tensor.matmul(
                            psq[:], wq_sb[:, kc * JC + l * 128: kc * JC + (l + 1) * 128],
                            hs_sb[kc][:, ts5], start=(kc == 0), stop=(kc == KC - 1))
                    qraw.append(rope_pre(psq[:], w5))
                if pend["fin"] is not None:
                    pend["fin"]()
                    pend["fin"] = None
                for l in range(HL):
                    rot = rope_rot(qraw[l], pool=pps, tag="ps")
                    rope_post(qraw[l], rot, qrT[l][:, ts5], c0, c1)
                if tb > 0:
                    o_proj(tb - 1)

                cam = []  # per-head deferred CaM state
                psavl = []
                recipl = []
                for l in range(HL):
                    E = []
                    for b in range(NB):
                        pst = pps.tile([128, w5], F32, tag="ps")
                        nc.tensor.matmul(pst[:], krT[l][:, b * 128:(b + 1) * 128],
                                         qrT[l][:, ts5], start=True, stop=True)
                        e = pe16.tile([128, w5], F16, tag="e")
                        nc.scalar.activation(e[:], pst[:], AF.Exp)
                        if b == NB - 1:
                            # drop the non-key row (perm col 1023) from softmax
                            nc.vector.tensor_scalar_mul(e[:], e[:],
                                                        masksf_sb[:, 0:1])
                        E.append(e)
                    psav = pav.tile([128, w5], F32, tag="av")
                    tail_it = (tb == len(TBS) - 1 and l == HL - 1)
                    if tail_it:
                        # denominator first: its recip chain overlaps the AV MMs
                        psdn = ppsd.tile([1, w5], F32, tag="dn")
                        for b in range(NB):
                            nc.tensor.matmul(psdn[:], ones[:], E[b][:],
                                             start=(b == 0), stop=(b == NB - 1))
                        dnf = None
                        recip = prow.tile([1, w5], F32, tag="row512")
                        nc.vector.reciprocal(recip[:], psdn[0:1, :])
                        rbf = ptmp.tile([128, w5], F32, tag="tmp")
                        nc.gpsimd.partition_broadcast(rbf[:], recip[:])
                        for b in range(NB):
                            nc.tensor.matmul(psav[:], vt[b][:, l * D:(l + 1) * D],
                                             E[b][:],
                                             start=(b == 0), stop=(b == NB - 1))
                    else:
                        esum = pe16.tile([128, w5], F16, tag="e")
                        nc.vector.tensor_add(esum[:], E[0][:], E[1][:])
                        for b in range(2, NB):
                            nc.vector.tensor_add(esum[:], esum[:], E[b][:])
                        dnf = ptmp.tile([128, w5], F32, tag="tmp")
                        nc.gpsimd.partition_all_reduce(dnf[:], esum[:], channels=128,
                                                       reduce_op=bass_isa.ReduceOp.add)
                        rbf = ptmp.tile([128, w5], F32, tag="tmp")
                        nc.vector.reciprocal(rbf[:], dnf[:])
                        for b in range(NB):
                            nc.tensor.matmul(psav[:], vt[b][:, l * D:(l + 1) * D],
                                             E[b][:],
                                             start=(b == 0), stop=(b == NB - 1))
                    if tb != CAMTB:
                        nc.vector.tensor_mul(outT[l][:, ts5], psav[:], rbf[:])
                    psavl.append(psav)
                    recipl.append((rbf, dnf))

                    if tb == CAMTB:
                        # ---- CaM scalar chain (PE part deferred past attn l1) ----
                        # sum of E over start+evict rows at CaM cols (255..511)
                        pssA = ppsd.tile([1, 256], F32, tag="dn")
                        nc.tensor.matmul(pssA[:], ones[:], E[0][:, 255:511],
                                         start=True, stop=False)
                        nc.tensor.matmul(pssA[:], mask77A, E[1][:, 255:511],
                                         start=False, stop=True)
                        pssA_sb = prow.tile([1, 256], F32, tag="row256")
                        nc.scalar.copy(pssA_sb[:], pssA[:])
                        # E row of evict key (perm col 204 = block 1 row 76)
                        erow16 = prow.tile([1, 256], F16, tag="row256h")
                        nc.gpsimd.dma_start(erow16[:], E[1][76:77, 255:511])
                        erow = prow.tile([1, 256], F32, tag="row256")
                        nc.vector.tensor_copy(erow[:], erow16[:])
                        srec = prow.tile([1, 256], F32, tag="row256")
                        nc.vector.tensor_sub(srec[:], dnf[0:1, 255:511], pssA_sb[:])
                        # scalars at t = 2047 (perm col 1022 = within-slice 255)
                        r_last = rbf[0:1, 510:511]
                        num = prow.tile([1, 1], F32, tag="sc")
                        nc.vector.tensor_mul(num[:], erow[0:1, 255:256], r_last)
                        mean = prow.tile([1, 1], F32, tag="sc")
                        nc.vector.tensor_mul(mean[:], srec[0:1, 255:256], r_last)
                        nc.vector.tensor_scalar_mul(mean[:], mean[:], 1.0 / 818.0)
                        nc.vector.tensor_scalar_add(mean[:], mean[:], 1e-6)
                        um = prow.tile([1, 1], F32, tag="sc")
                        nc.vector.tensor_mul(um[:], u2_sb[0:1, l:l + 1], mean[:])
                        bern = prow.tile([1, 1], F32, tag="sc")
                        nc.vector.tensor_tensor(bern[:], um[:], num[:],
                                                mybir.AluOpType.is_lt)
                        bs = prow.tile([1, 1], F32, tag="sc")
                        nc.vector.tensor_scalar_mul(bs[:], bern[:], 1.0 / RB)
                        coef_f = prow.tile([1, 256], F32, tag="row256")
                        nc.vector.tensor_scalar_mul(coef_f[:], srec[:], bs[:])
                        coef = prow.tile([1, 256], F16, tag="row256h")
                        nc.vector.tensor_copy(coef[:], coef_f[:])
                        vrow = prow.tile([1, D], F16, tag="vrow")
                        nc.gpsimd.dma_start(vrow[:], vt[1][76:77, l * D:(l + 1) * D])
                        cam.append((coef, vrow))

                # normalize; at tb==CAMTB defer (incl. CaM rank-1) into next tb's
                # slot so the CaM scalar chain never head-blocks the PE queue
                if tb == CAMTB:
                    def fin(ts5=ts5, cam=cam, psavl=psavl, recipl=recipl):
                        for l in range(HL):
                            psav, (rbf, _dnf) = psavl[l], recipl[l]
                            coef, vrow = cam[l]
                            pscr = pps.tile([128, 256], F32, tag="ps")
                            nc.tensor.matmul(pscr[:], vrow[:], coef[:],
                                             start=True, stop=True)
                            nout = ptmp.tile([128, 512], F32, tag="tmp")
                            nc.vector.tensor_mul(nout[:], psav[:], rbf[:])
                            corr = ptmp.tile([128, 256], F32, tag="tmp")
                            nc.vector.tensor_mul(corr[:], pscr[:], rbf[:, 255:511])
                            nc.vector.tensor_add(nout[:, 255:511],
                                                 nout[:, 255:511], corr[:])
                            nc.vector.tensor_copy(outT[l][:, ts5], nout[:])
                    pend["fin"] = fin


            o_proj(len(TBS) - 1)

    nc.compile()
    return nc


_NC_CACHE = None


def _get_nc():
    global _NC_CACHE
    if _NC_CACHE is None:
        _NC_CACHE = _build_nc()
    return _NC_CACHE


PERM = np.concatenate([np.arange(0, SB), np.arange(T - RB, T),
                       np.arange(SB, T - RB)])


def make_in_maps(hidden_states, Wq, Wk, Wv, Wo):
    hs = np.asarray(hidden_states, np.float32).reshape(T, DM)
    hs = np.nan_to_num(hs, nan=0.0, posinf=1e4, neginf=-1e4)
    hsT = np.ascontiguousarray(hs.T[:, PERM].astype(np.float16))
    Wq = np.asarray(Wq, np.float32)
    Wk = np.asarray(Wk, np.float32)
    Wv = np.asarray(Wv, np.float32)
    Wo = np.asarray(Wo, np.float32)

    inv_freq = 1.0 / (10000.0 ** (np.arange(0, D, 2, dtype=np.float32) / D))
    freqs = np.arange(T, dtype=np.float32)[:, None] * inv_freq[None, :]
    emb = np.concatenate([freqs, freqs], axis=-1)          # [T, D]
    cosT = np.ascontiguousarray(np.cos(emb).T[:, PERM].astype(np.float32))
    sinTs = np.ascontiguousarray(np.sin(emb).T[:, PERM].astype(np.float32))
    # rotate-half as a PE stationary: rot(x)[i] = -x[i+64] (i<64), x[i-64] (else)
    prot = np.zeros((D, D), np.float32)
    for i in range(64):
        prot[i + 64, i] = -1.0
        prot[i, i + 64] = 1.0

    import jax
    import jax.numpy as jnp
    u_full = np.asarray(
        jax.random.uniform(jax.random.key(42), (1, H), jnp.float32))

    maskh = np.zeros((128, 2), np.float16)
    maskh[:127, 0] = 1.0      # maskB16: drop row 127 of key block 7
    maskh[:77, 1] = 1.0       # mask77A: start+evict rows of key block 1
    maskf = np.zeros((128, 1), np.float32)
    maskf[:127, 0] = 1.0

    scale = 1.0 / np.sqrt(np.float32(D))

    def wlayout(wT):
        # wT: [DM, JC] -> SBUF layout [128, KC*JC]: [p, kc*JC + j]
        return np.ascontiguousarray(
            wT.reshape(KC, 128, JC).transpose(1, 0, 2).reshape(128, KC * JC)
            .astype(np.float16))

    in_maps = []
    for c in range(NCORES):
        js = slice(c * JC, (c + 1) * JC)
        woT = Wo[:, js].T                                  # [JC, DM]
        wo_l = np.ascontiguousarray(
            woT.reshape(HL, 128, DM).transpose(1, 0, 2).reshape(128, HL * DM)
            .astype(np.float16))
        in_maps.append({
            "hsT": hsT,
            "wq": wlayout(Wq[js, :].T * scale),
            "wk": wlayout(Wk[js, :].T),
            "wv": wlayout(Wv[js, :].T),
            "wo": wo_l,
            "cosT": cosT,
            "sinTs": sinTs,
            "protT": prot,
            "u2": np.ascontiguousarray(u_full[:, c * HL:(c + 1) * HL]),
            "masksh": maskh,
            "masksf": maskf,
        })
    return in_maps


def kernel(hidden_states, Wq, Wk, Wv, Wo):
    nc = _get_nc()
    in_maps = make_in_maps(hidden_states, Wq, Wk, Wv, Wo)
    res = bass_utils.run_bass_kernel_spmd(nc, in_maps,
                                          core_ids=list(range(NCORES)))
    acc = np.zeros((T, DM), np.float32)
    for c in range(NCORES):
        acc += res.results[c]["po"].astype(np.float32)
    out = np.empty((T, DM), np.float32)
    out[PERM] = acc                                       # undo token permutation
    out = np.nan_to_num(out, nan=0.0, posinf=1e4, neginf=-1e4)
    return out.reshape(1, T, DM)


# revision 42
# speedup vs baseline: 40090.9866x; 1.0299x over previous
"""Trainium2 Bass kernel for nn_LlamaAttention_cam (sparse attention + CaM merge).

Sharding: tensor-parallel over heads across 8 NeuronCores (2 heads/core).
Each core computes its heads' QKV projections, RoPE, masked attention
(start+recent keep mask), CaM rank-1 correction for the last chunk, and a
partial o_proj.  The host sums the 8 partial outputs (the reduction of the
head-parallel o_proj), replacing the all-reduce.

Token positions are permuted host-side to [start | recent | rest] so the
1023 kept keys occupy the first 1023 columns: key blocks are 8 full
128-blocks (block 7 has one non-key token, masked via an indicator
stationary + a zeroed V row).  The CaM chunk (t in [1792,2048)) lands in
t-block 1 at columns 767..1022, so the serial CaM chain overlaps the rest
of the pipeline instead of sitting in the tail.  hs streams in two waves
(kept tokens first) so K/V projections start early.  o_proj is software-
pipelined one t-block behind attention.  All matmul IO is fp16; PSUM
accumulation fp32; softmax/CaM scalar math fp32.
"""

import sys

for _p in ("/opt/trn_rl_repo",):
    if _p not in sys.path:
        sys.path.append(_p)

import numpy as np

import concourse.bass as bass
import concourse.bass_isa as bass_isa
import concourse.mybir as mybir
import concourse.tile as tile
from concourse import bacc, bass_utils

F32 = mybir.dt.float32
F16 = mybir.dt.float16
AF = mybir.ActivationFunctionType

T = 2048
DM = 2048
H = 16
D = 128
NCORES = 8
HL = H // NCORES          # heads per core = 2
JC = HL * D               # local attn width = 256
SB = 204                  # start keep
RB = 819                  # recent keep
KC = DM // 128            # 16 model-dim chunks
TB = T // 512             # 4 t-blocks of 512
TBS = [(0, 512), (512, 1024), (1024, 1536), (1536, 2048)]
NB = 8                    # kept-key blocks (perm cols 0..1024, col 1023 masked)
# perm order: [0..204) + [1229..2048) + [204..1229)
# CaM: evict key 1229 -> perm col 204 (block 1, row 76)
# CaM q range t in [1792,2048) -> perm cols 767..1022 = tb1 local cols 255..510
CAMTB = 1


def _build_nc():
    nc = bacc.Bacc("TRN2", target_bir_lowering=False, debug=False,
                   num_devices=NCORES)
    hsT = nc.dram_tensor("hsT", [DM, T], F16, kind="ExternalInput").ap()
    wq = nc.dram_tensor("wq", [128, KC * JC], F16, kind="ExternalInput").ap()
    wk = nc.dram_tensor("wk", [128, KC * JC], F16, kind="ExternalInput").ap()
    wv = nc.dram_tensor("wv", [128, KC * JC], F16, kind="ExternalInput").ap()
    wo = nc.dram_tensor("wo", [128, HL * DM], F16, kind="ExternalInput").ap()
    cosT = nc.dram_tensor("cosT", [D, T], F32, kind="ExternalInput").ap()
    sinTs = nc.dram_tensor("sinTs", [D, T], F32, kind="ExternalInput").ap()
    protT = nc.dram_tensor("protT", [D, D],
                           mybir.dt.float32r, kind="ExternalInput").ap()
    u2 = nc.dram_tensor("u2", [1, HL], F32, kind="ExternalInput").ap()
    masksh = nc.dram_tensor("masksh", [128, 2], F16, kind="ExternalInput").ap()
    masksf = nc.dram_tensor("masksf", [128, 1], F32, kind="ExternalInput").ap()
    po = nc.dram_tensor("po", [T, DM], F16, kind="ExternalOutput").ap()

    with tile.TileContext(nc) as tc:
        with (
            tc.tile_pool(name="resid", bufs=1) as pres,        # long-lived
            tc.tile_pool(name="e16", bufs=12) as pe16,         # f16 [128,512] E tiles
            tc.tile_pool(name="tmp", bufs=10) as ptmp,          # f32 [128,512] transients
            tc.tile_pool(name="post", bufs=4) as ppost,        # f16 [128,2048] out staging
            tc.tile_pool(name="rows", bufs=4) as prow,         # small [1,*] tiles
            tc.tile_pool(name="ps", bufs=4, space="PSUM") as pps,
            tc.tile_pool(name="psav", bufs=2, space="PSUM") as pav,
            tc.tile_pool(name="psdn", bufs=2, space="PSUM") as ppsd,
        ):
            # ---- input DMAs: wk first, then kept-token hs wave, then the rest ----
            wk_sb = pres.tile([128, KC * JC], F16, tag="wk")
            hs_sb = [pres.tile([128, T], F16, tag=f"hs{kc}", name=f"hs{kc}")
                     for kc in range(KC)]
            wv_sb = pres.tile([128, KC * JC], F16, tag="wv")
            nc.sync.dma_start(wk_sb[:, 0:512], wk[:, 0:512])
            nc.sync.dma_start(hs_sb[0][:, 0:512], hsT[0:128, 0:512])
            nc.sync.dma_start(hs_sb[0][:, 512:1024], hsT[0:128, 512:1024])
            nc.sync.dma_start(hs_sb[1][:, 0:1024], hsT[128:256, 0:1024])
            nc.sync.dma_start(wk_sb[:, 512:1024], wk[:, 512:1024])
            for kc in range(2, 4):
                nc.sync.dma_start(hs_sb[kc][:, 0:1024],
                                  hsT[kc * 128:(kc + 1) * 128, 0:1024])
            nc.sync.dma_start(wk_sb[:, 1024:2048], wk[:, 1024:2048])
            for kc in range(4, 8):
                nc.sync.dma_start(hs_sb[kc][:, 0:1024],
                                  hsT[kc * 128:(kc + 1) * 128, 0:1024])
            nc.sync.dma_start(wk_sb[:, 2048:4096], wk[:, 2048:4096])
            for kc in range(8, KC):
                nc.sync.dma_start(hs_sb[kc][:, 0:1024],
                                  hsT[kc * 128:(kc + 1) * 128, 0:1024])
            for qq in range(4):
                nc.sync.dma_start(wv_sb[:, qq * 1024:(qq + 1) * 1024],
                                  wv[:, qq * 1024:(qq + 1) * 1024])
            cosT_sb = pres.tile([D, T], F32, tag="cos")
            sinTs_sb = pres.tile([D, T], F32, tag="sin")
            protT_sb = pres.tile([D, D], mybir.dt.float32r, tag="prot")
            nc.sync.dma_start(cosT_sb[:], cosT[:])
            nc.sync.dma_start(sinTs_sb[:], sinTs[:])
            nc.sync.dma_start(protT_sb[:], protT[:])
            wq_sb = pres.tile([128, KC * JC], F16, tag="wq")
            nc.sync.dma_start(wq_sb[:], wq[:])
            for kc in range(KC):
                nc.sync.dma_start(hs_sb[kc][:, 1024:2048],
                                  hsT[kc * 128:(kc + 1) * 128, 1024:2048])
            wo_sb = pres.tile([128, HL * DM], F16, tag="wo")
            nc.sync.dma_start(wo_sb[:], wo[:])
            u2_sb = pres.tile([1, HL], F32, tag="u2")
            nc.sync.dma_start(u2_sb[:], u2[:])
            masksh_sb = pres.tile([128, 2], F16, tag="masksh")
            nc.sync.dma_start(masksh_sb[:], masksh[:])
            masksf_sb = pres.tile([128, 1], F32, tag="masksf")
            nc.sync.dma_start(masksf_sb[:], masksf[:])
            maskB16 = masksh_sb[:, 0:1]    # rows <= 126 (drop key col 1023)
            mask77A = masksh_sb[:, 1:2]    # rows <= 76 (start + evict rows of blk 1)

            ones_f = pres.tile([128, 1], F32, tag="ones_f")
            nc.vector.memset(ones_f[:], 1.0)
            ones = pres.tile([128, 1], F16, tag="ones")
            nc.vector.tensor_copy(ones[:], ones_f[:])

            # rope'd q/k in [d, t] layout; v in [t, d_local] layout
            qrT = [pres.tile([D, T], F16, tag=f"qrT{l}", name=f"qrT{l}") for l in range(HL)]
            krT = [pres.tile([D, T], F16, tag=f"krT{l}", name=f"krT{l}") for l in range(HL)]
            vt = [pres.tile([128, JC], F16, tag=f"vt{b}", name=f"vt{b}") for b in range(NB)]
            outT = [pres.tile([D, T], F16, tag=f"outT{l}", name=f"outT{l}") for l in range(HL)]

            F32R = mybir.dt.float32r

            def rope_pre(ps_ap, w):
                raw = ptmp.tile([128, w], F32R, tag="tmp")
                nc.scalar.copy(raw[:], ps_ap)
                return raw

            def rope_rot(raw, pool=None, tag="av"):
                rot = (pool or pav).tile([128, raw.shape[1]], F32, tag=tag)
                nc.tensor.matmul(rot[:], protT_sb[:], raw[:],
                                 start=True, stop=True)
                return rot

            def rope_post(raw, rot, dst_ap, c0, c1):
                w = c1 - c0
                t1 = ptmp.tile([128, w], F32, tag="tmp")
                nc.gpsimd.tensor_mul(t1[:], raw[:].bitcast(F32), cosT_sb[:, c0:c1])
                t2 = ptmp.tile([128, w], F32, tag="tmp")
                nc.vector.tensor_mul(t2[:], rot[:], sinTs_sb[:, c0:c1])
                nc.vector.tensor_add(dst_ap, t1[:], t2[:])

            # ---------------- phase K: kept keys (perm cols 0..1024), kc-outer ----
            psk = []
            for l in range(HL):
                pska = pps.tile([128, 512], F32, tag="ps", name=f"pska{l}")
                pskb = pps.tile([128, 512], F32, tag="ps", name=f"pskb{l}")
                psk.append((pska, pskb))
            for kc in range(KC):
                for l in range(HL):
                    wsl = wk_sb[:, kc * JC + l * 128: kc * JC + (l + 1) * 128]
                    nc.tensor.matmul(psk[l][0][:], wsl, hs_sb[kc][:, 0:512],
                                     start=(kc == 0), stop=(kc == KC - 1))
                    nc.tensor.matmul(psk[l][1][:], wsl, hs_sb[kc][:, 512:1024],
                                     start=(kc == 0), stop=(kc == KC - 1))
            kraw = []
            for l in range(HL):
                kraw.append((rope_pre(psk[l][0][:], 512),
                             rope_pre(psk[l][1][:], 512)))

            # ---------------- phase V (paired blocks, issued after q-tb0) ----------
            def v_phase():
                for bp in range(0, NB, 2):
                    if bp == 2:
                        for l in range(HL):
                            ra, rb = kraw[l]
                            rota = rope_rot(ra)
                            rope_post(ra, rota, krT[l][:, 0:512], 0, 512)
                            rotb = rope_rot(rb)
                            rope_post(rb, rotb, krT[l][:, 512:1024], 512, 1024)
                    psvA = pps.tile([128, JC], F32, tag="ps", name=f"psvA{bp}")
                    psvB = pps.tile([128, JC], F32, tag="ps", name=f"psvB{bp}")
                    for kc in range(KC):
                        nc.tensor.matmul(psvA[:], hs_sb[kc][:, bp * 128:(bp + 1) * 128],
                                         wv_sb[:, kc * JC:(kc + 1) * JC],
                                         start=(kc == 0), stop=(kc == KC - 1))
                        nc.tensor.matmul(psvB[:],
                                         hs_sb[kc][:, (bp + 1) * 128:(bp + 2) * 128],
                                         wv_sb[:, kc * JC:(kc + 1) * JC],
                                         start=(kc == 0), stop=(kc == KC - 1))
                    nc.scalar.copy(vt[bp][:], psvA[:])
                    nc.vector.tensor_copy(vt[bp + 1][:], psvB[:])
                    if bp + 1 == NB - 1:
                        # zero the non-key row (perm col 1023) so AV ignores it
                        nc.vector.tensor_scalar_mul(vt[bp + 1][:], vt[bp + 1][:],
                                                    masksf_sb[:, 0:1])

            # ---------------- phase Q + attention + pipelined o_proj ----------------
            def o_proj(tb):
                c0, c1 = TBS[tb]
                for ti in range(c0 // 128, c1 // 128):
                    last = (ti == 4 * TB - 1)
                    postg = ppost.tile([128, DM], F16, tag="post")
                    for mb in range(TB):
                        if mb < 2:
                            pso = pps.tile([128, 512], F32, tag="ps")
                        else:
                            pso = pav.tile([128, 512], F32, tag="av")
                        for l in range(HL):
                            nc.tensor.matmul(
                                pso[:], outT[l][:, ti * 128:(ti + 1) * 128],
                                wo_sb[:, l * DM + mb * 512: l * DM + (mb + 1) * 512],
                                start=(l == 0), stop=(l == HL - 1))
                        if mb % 2 == 0:
                            nc.scalar.copy(postg[:, mb * 512:(mb + 1) * 512], pso[:])
                        else:
                            nc.vector.tensor_copy(postg[:, mb * 512:(mb + 1) * 512],
                                                  pso[:])
                        if last:
                            eng = nc.scalar if mb % 2 == 0 else nc.gpsimd
                            eng.dma_start(
                                po[ti * 128:(ti + 1) * 128,
                                   mb * 512:(mb + 1) * 512],
                                postg[:, mb * 512:(mb + 1) * 512])
                    if not last:
                        nc.sync.dma_start(po[ti * 128:(ti + 1) * 128, :], postg[:])

            v_phase()
            pend = {"fin": None}

            for tb in range(len(TBS)):
                c0, c1 = TBS[tb]
                w5 = c1 - c0
                ts5 = slice(c0, c1)
                qraw = []
                for l in range(HL):
                    psq = pps.tile([128, w5], F32, tag="ps")
                    for kc in range(KC):
                        nc.tensor.matmul(
                            psq[:], wq_sb[:, kc * JC + l * 128: kc * JC + (l + 1) * 128],
                            hs_sb[kc][:, ts5], start=(kc == 0), stop=(kc == KC - 1))
                    qraw.append(rope_pre(psq[:], w5))
                if pend["fin"] is not None:
                    pend["fin"]()
                    pend["fin"] = None
                for l in range(HL):
                    rot = rope_rot(qraw[l], pool=pps, tag="ps")
                    rope_post(qraw[l], rot, qrT[l][:, ts5], c0, c1)
                if tb > 0:
                    o_proj(tb - 1)

                cam = []  # per-head deferred CaM state
                psavl = []
                recipl = []
                for l in range(HL):
                    E = []
                    for b in range(NB):
                        pst = pps.tile([128, w5], F32, tag="ps")
                        nc.tensor.matmul(pst[:], krT[l][:, b * 128:(b + 1) * 128],
                                         qrT[l][:, ts5], start=True, stop=True)
                        e = pe16.tile([128, w5], F16, tag="e")
                        nc.scalar.activation(e[:], pst[:], AF.Exp)
                        if b == NB - 1:
                            # drop the non-key row (perm col 1023) from softmax
                            nc.vector.tensor_scalar_mul(e[:], e[:],
                                                        masksf_sb[:, 0:1])
                        E.append(e)
                    psav = pav.tile([128, w5], F32, tag="av")
                    tail_it = (tb == len(TBS) - 1 and l == HL - 1)
                    if tail_it:
                        # denominator first: its recip chain overlaps the AV MMs
                        psdn = ppsd.tile([1, w5], F32, tag="dn")
                        for b in range(NB):
                            nc.tensor.matmul(psdn[:], ones[:], E[b][:],
                                             start=(b == 0), stop=(b == NB - 1))
                        dnf = None
                        recip = prow.tile([1, w5], F32, tag="row512")
                        nc.vector.reciprocal(recip[:], psdn[0:1, :])
                        rbf = ptmp.tile([128, w5], F32, tag="tmp")
                        nc.gpsimd.partition_broadcast(rbf[:], recip[:])
                        for b in range(NB):
                            nc.tensor.matmul(psav[:], vt[b][:, l * D:(l + 1) * D],
                                             E[b][:],
                                             start=(b == 0), stop=(b == NB - 1))
                    else:
                        esum = pe16.tile([128, w5], F16, tag="e")
                        nc.vector.tensor_add(esum[:], E[0][:], E[1][:])
                        for b in range(2, NB):
                            nc.vector.tensor_add(esum[:], esum[:], E[b][:])
                        dnf = ptmp.tile([128, w5], F32, tag="tmp")
                        nc.gpsimd.partition_all_reduce(dnf[:], esum[:], channels=128,
                                                       reduce_op=bass_isa.ReduceOp.add)
                        rbf = ptmp.tile([128, w5], F32, tag="tmp")
                        nc.vector.reciprocal(rbf[:], dnf[:])
                        for b in range(NB):
                            nc.tensor.matmul(psav[:], vt[b][:, l * D:(l + 1) * D],
                                             E[b][:],
                                             start=(b == 0), stop=(b == NB - 1))
                    if tb != CAMTB:
                        nc.vector.tensor_mul(outT[l][:, ts5], psav[:], rbf[:])
                    psavl.append(psav)
                    recipl.append((rbf, dnf))

                    if tb == CAMTB:
                        # ---- CaM scalar chain (PE part deferred past attn l1) ----
                        # sum of E over start+evict rows at CaM cols (255..511)
                        pssA = ppsd.tile([1, 256], F32, tag="dn")
                        nc.tensor.matmul(pssA[:], ones[:], E[0][:, 255:511],
                                         start=True, stop=False)
                        nc.tensor.matmul(pssA[:], mask77A, E[1][:, 255:511],
                                         start=False, stop=True)
                        pssA_sb = prow.tile([1, 256], F32, tag="row256")
                        nc.scalar.copy(pssA_sb[:], pssA[:])
                        # E row of evict key (perm col 204 = block 1 row 76)
                        erow16 = prow.tile([1, 256], F16, tag="row256h")
                        nc.gpsimd.dma_start(erow16[:], E[1][76:77, 255:511])
                        erow = prow.tile([1, 256], F32, tag="row256")
                        nc.vector.tensor_copy(erow[:], erow16[:])
                        srec = prow.tile([1, 256], F32, tag="row256")
                        nc.vector.tensor_sub(srec[:], dnf[0:1, 255:511], pssA_sb[:])
                        # scalars at t = 2047 (perm col 1022 = within-slice 255)
                        r_last = rbf[0:1, 510:511]
                        num = prow.tile([1, 1], F32, tag="sc")
                        nc.vector.tensor_mul(num[:], erow[0:1, 255:256], r_last)
                        mean = prow.tile([1, 1], F32, tag="sc")
                        nc.vector.tensor_mul(mean[:], srec[0:1, 255:256], r_last)
                        nc.vector.tensor_scalar_mul(mean[:], mean[:], 1.0 / 818.0)
                        nc.vector.tensor_scalar_add(mean[:], mean[:], 1e-6)
                        um = prow.tile([1, 1], F32, tag="sc")
                        nc.vector.tensor_mul(um[:], u2_sb[0:1, l:l + 1], mean[:])
                        bern = prow.tile([1, 1], F32, tag="sc")
                        nc.vector.tensor_tensor(bern[:], um[:], num[:],
                                                mybir.AluOpType.is_lt)
                        bs = prow.tile([1, 1], F32, tag="sc")
                        nc.vector.tensor_scalar_mul(bs[:], bern[:], 1.0 / RB)
                        coef_f = prow.tile([1, 256], F32, tag="row256")
                        nc.vector.tensor_scalar_mul(coef_f[:], srec[:], bs[:])
                        coef = prow.tile([1, 256], F16, tag="row256h")
                        nc.vector.tensor_copy(coef[:], coef_f[:])
                        vrow = prow.tile([1, D], F16, tag="vrow")
                        nc.gpsimd.dma_start(vrow[:], vt[1][76:77, l * D:(l + 1) * D])
                        cam.append((coef, vrow))

                # normalize; at tb==CAMTB defer (incl. CaM rank-1) into next tb's
                # slot so the CaM scalar chain never head-blocks the PE queue
                if tb == CAMTB:
                    def fin(ts5=ts5, cam=cam, psavl=psavl, recipl=recipl):
                        for l in range(HL):
                            psav, (rbf, _dnf) = psavl[l], recipl[l]
                            coef, vrow = cam[l]
                            pscr = pps.tile([128, 256], F32, tag="ps")
                            nc.tensor.matmul(pscr[:], vrow[:], coef[:],
                                             start=True, stop=True)
                            nout = ptmp.tile([128, 512], F32, tag="tmp")
                            nc.vector.tensor_mul(nout[:], psav[:], rbf[:])
                            corr = ptmp.tile([128, 256], F32, tag="tmp")
                            nc.vector.tensor_mul(corr[:], pscr[:], rbf[:, 255:511])
                            nc.vector.tensor_add(nout[:, 255:511],
                                                 nout[:, 255:511], corr[:])
                            nc.vector.tensor_copy(outT[l][:, ts5], nout[:])
                    pend["fin"] = fin


            o_proj(len(TBS) - 1)

    nc.compile()
    return nc


_NC_CACHE = None


def _get_nc():
    global _NC_CACHE
    if _NC_CACHE is None:
        _NC_CACHE = _build_nc()
    return _NC_CACHE


PERM = np.concatenate([np.arange(0, SB), np.arange(T - RB, T),
                       np.arange(SB, T - RB)])


def make_in_maps(hidden_states, Wq, Wk, Wv, Wo):
    hs = np.asarray(hidden_states, np.float32).reshape(T, DM)
    hs = np.nan_to_num(hs, nan=0.0, posinf=1e4, neginf=-1e4)
    hsT = np.ascontiguousarray(hs.T[:, PERM].astype(np.float16))
    Wq = np.asarray(Wq, np.float32)
    Wk = np.asarray(Wk, np.float32)
    Wv = np.asarray(Wv, np.float32)
    Wo = np.asarray(Wo, np.float32)

    inv_freq = 1.0 / (10000.0 ** (np.arange(0, D, 2, dtype=np.float32) / D))
    freqs = np.arange(T, dtype=np.float32)[:, None] * inv_freq[None, :]
    emb = np.concatenate([freqs, freqs], axis=-1)          # [T, D]
    cosT = np.ascontiguousarray(np.cos(emb).T[:, PERM].astype(np.float32))
    sinTs = np.ascontiguousarray(np.sin(emb).T[:, PERM].astype(np.float32))
    # rotate-half as a PE stationary: rot(x)[i] = -x[i+64] (i<64), x[i-64] (else)
    prot = np.zeros((D, D), np.float32)
    for i in range(64):
        prot[i + 64, i] = -1.0
        prot[i, i + 64] = 1.0

    import jax
    import jax.numpy as jnp
    u_full = np.asarray(
        jax.random.uniform(jax.random.key(42), (1, H), jnp.float32))

    maskh = np.zeros((128, 2), np.float16)
    maskh[:127, 0] = 1.0      # maskB16: drop row 127 of key block 7
    maskh[:77, 1] = 1.0       # mask77A: start+evict rows of key block 1
    maskf = np.zeros((128, 1), np.float32)
    maskf[:127, 0] = 1.0

    scale = 1.0 / np.sqrt(np.float32(D))

    def wlayout(wT):
        # wT: [DM, JC] -> SBUF layout [128, KC*JC]: [p, kc*JC + j]
        return np.ascontiguousarray(
            wT.reshape(KC, 128, JC).transpose(1, 0, 2).reshape(128, KC * JC)
            .astype(np.float16))

    in_maps = []
    for c in range(NCORES):
        js = slice(c * JC, (c + 1) * JC)
        woT = Wo[:, js].T                                  # [JC, DM]
        wo_l = np.ascontiguousarray(
            woT.reshape(HL, 128, DM).transpose(1, 0, 2).reshape(128, HL * DM)
            .astype(np.float16))
        in_maps.append({
            "hsT": hsT,
            "wq": wlayout(Wq[js, :].T * scale),
            "wk": wlayout(Wk[js, :].T),
            "wv": wlayout(Wv[js, :].T),
            "wo": wo_l,
            "cosT": cosT,
            "sinTs": sinTs,
            "protT": prot,
            "u2": np.ascontiguousarray(u_full[:, c * HL:(c + 1) * HL]),
            "masksh": maskh,
            "masksf": maskf,
        })
    return in_maps


def kernel(hidden_states, Wq, Wk, Wv, Wo):
    nc = _get_nc()
    in_maps = make_in_maps(hidden_states, Wq, Wk, Wv, Wo)
    res = bass_utils.run_bass_kernel_spmd(nc, in_maps,
                                          core_ids=list(range(NCORES)))
    acc = np.zeros((T, DM), np.float32)
    for c in range(NCORES):
        acc += res.results[c]["po"].astype(np.float32)
    out = np.empty((T, DM), np.float32)
    out[PERM] = acc                                       # undo token permutation
    out = np.nan_to_num(out, nan=0.0, posinf=1e4, neginf=-1e4)
    return out.reshape(1, T, DM)


# revision 47
# speedup vs baseline: 40138.4734x; 1.0012x over previous
"""Trainium2 Bass kernel for nn_LlamaAttention_cam (sparse attention + CaM merge).

Sharding: tensor-parallel over heads across 8 NeuronCores (2 heads/core).
Each core computes its heads' QKV projections, RoPE, masked attention
(start+recent keep mask), CaM rank-1 correction for the last chunk, and a
partial o_proj.  The host sums the 8 partial outputs (the reduction of the
head-parallel o_proj), replacing the all-reduce.

Token positions are permuted host-side to [start | recent | rest] so the
1023 kept keys occupy the first 1023 columns: key blocks are 8 full
128-blocks (block 7 has one non-key token, masked via an indicator
stationary + a zeroed V row).  The CaM chunk (t in [1792,2048)) lands in
t-block 1 at columns 767..1022, so the serial CaM chain overlaps the rest
of the pipeline instead of sitting in the tail.  hs streams in two waves
(kept tokens first) so K/V projections start early.  o_proj is software-
pipelined one t-block behind attention.  All matmul IO is fp16; PSUM
accumulation fp32; softmax/CaM scalar math fp32.
"""

import sys

for _p in ("/opt/trn_rl_repo",):
    if _p not in sys.path:
        sys.path.append(_p)

import numpy as np

import concourse.bass as bass
import concourse.bass_isa as bass_isa
import concourse.mybir as mybir
import concourse.tile as tile
from concourse import bacc, bass_utils

F32 = mybir.dt.float32
F16 = mybir.dt.float16
AF = mybir.ActivationFunctionType

T = 2048
DM = 2048
H = 16
D = 128
NCORES = 8
HL = H // NCORES          # heads per core = 2
JC = HL * D               # local attn width = 256
SB = 204                  # start keep
RB = 819                  # recent keep
KC = DM // 128            # 16 model-dim chunks
TB = T // 512             # 4 t-blocks of 512
TBS = [(0, 512), (512, 1024), (1024, 1536), (1536, 2048)]
NB = 8                    # kept-key blocks (perm cols 0..1024, col 1023 masked)
# perm order: [0..204) + [1229..2048) + [204..1229)
# CaM: evict key 1229 -> perm col 204 (block 1, row 76)
# CaM q range t in [1792,2048) -> perm cols 767..1022 = tb1 local cols 255..510
CAMTB = 1


def _build_nc():
    nc = bacc.Bacc("TRN2", target_bir_lowering=False, debug=False,
                   num_devices=NCORES)
    hsT = nc.dram_tensor("hsT", [DM, T], F16, kind="ExternalInput").ap()
    wq = nc.dram_tensor("wq", [128, KC * JC], F16, kind="ExternalInput").ap()
    wk = nc.dram_tensor("wk", [128, KC * JC], F16, kind="ExternalInput").ap()
    wv = nc.dram_tensor("wv", [128, KC * JC], F16, kind="ExternalInput").ap()
    wo = nc.dram_tensor("wo", [128, HL * DM], F16, kind="ExternalInput").ap()
    cosT = nc.dram_tensor("cosT", [D, T], F32, kind="ExternalInput").ap()
    sinTs = nc.dram_tensor("sinTs", [D, T], F32, kind="ExternalInput").ap()
    protT = nc.dram_tensor("protT", [D, D],
                           mybir.dt.float32r, kind="ExternalInput").ap()
    u2 = nc.dram_tensor("u2", [1, HL], F32, kind="ExternalInput").ap()
    masksh = nc.dram_tensor("masksh", [128, 2], F16, kind="ExternalInput").ap()
    masksf = nc.dram_tensor("masksf", [128, 1], F32, kind="ExternalInput").ap()
    po = nc.dram_tensor("po", [T, DM], F16, kind="ExternalOutput").ap()

    with tile.TileContext(nc) as tc:
        with (
            tc.tile_pool(name="resid", bufs=1) as pres,        # long-lived
            tc.tile_pool(name="e16", bufs=12) as pe16,         # f16 [128,512] E tiles
            tc.tile_pool(name="tmp", bufs=10) as ptmp,          # f32 [128,512] transients
            tc.tile_pool(name="post", bufs=4) as ppost,        # f16 [128,2048] out staging
            tc.tile_pool(name="rows", bufs=4) as prow,         # small [1,*] tiles
            tc.tile_pool(name="ps", bufs=4, space="PSUM") as pps,
            tc.tile_pool(name="psav", bufs=2, space="PSUM") as pav,
            tc.tile_pool(name="psdn", bufs=2, space="PSUM") as ppsd,
        ):
            # ---- input DMAs: wk first, then kept-token hs wave, then the rest ----
            wk_sb = pres.tile([128, KC * JC], F16, tag="wk")
            hs_sb = [pres.tile([128, T], F16, tag=f"hs{kc}", name=f"hs{kc}")
                     for kc in range(KC)]
            wv_sb = pres.tile([128, KC * JC], F16, tag="wv")
            nc.sync.dma_start(wk_sb[:, 0:512], wk[:, 0:512])
            nc.scalar.dma_start(hs_sb[0][:, 0:512], hsT[0:128, 0:512])
            nc.scalar.dma_start(hs_sb[0][:, 512:1024], hsT[0:128, 512:1024])
            nc.sync.dma_start(hs_sb[1][:, 0:1024], hsT[128:256, 0:1024])
            nc.sync.dma_start(wk_sb[:, 512:1024], wk[:, 512:1024])
            for kc in range(2, 4):
                nc.sync.dma_start(hs_sb[kc][:, 0:1024],
                                  hsT[kc * 128:(kc + 1) * 128, 0:1024])
            nc.sync.dma_start(wk_sb[:, 1024:2048], wk[:, 1024:2048])
            for kc in range(4, 8):
                nc.sync.dma_start(hs_sb[kc][:, 0:1024],
                                  hsT[kc * 128:(kc + 1) * 128, 0:1024])
            nc.sync.dma_start(wk_sb[:, 2048:4096], wk[:, 2048:4096])
            for kc in range(8, KC):
                nc.sync.dma_start(hs_sb[kc][:, 0:1024],
                                  hsT[kc * 128:(kc + 1) * 128, 0:1024])
            for qq in range(4):
                nc.sync.dma_start(wv_sb[:, qq * 1024:(qq + 1) * 1024],
                                  wv[:, qq * 1024:(qq + 1) * 1024])
            cosT_sb = pres.tile([D, T], F32, tag="cos")
            sinTs_sb = pres.tile([D, T], F32, tag="sin")
            protT_sb = pres.tile([D, D], mybir.dt.float32r, tag="prot")
            nc.sync.dma_start(cosT_sb[:], cosT[:])
            nc.sync.dma_start(sinTs_sb[:], sinTs[:])
            nc.sync.dma_start(protT_sb[:], protT[:])
            wq_sb = pres.tile([128, KC * JC], F16, tag="wq")
            nc.sync.dma_start(wq_sb[:], wq[:])
            for kc in range(KC):
                nc.sync.dma_start(hs_sb[kc][:, 1024:2048],
                                  hsT[kc * 128:(kc + 1) * 128, 1024:2048])
            wo_sb = pres.tile([128, HL * DM], F16, tag="wo")
            nc.sync.dma_start(wo_sb[:], wo[:])
            u2_sb = pres.tile([1, HL], F32, tag="u2")
            nc.sync.dma_start(u2_sb[:], u2[:])
            masksh_sb = pres.tile([128, 2], F16, tag="masksh")
            nc.sync.dma_start(masksh_sb[:], masksh[:])
            masksf_sb = pres.tile([128, 1], F32, tag="masksf")
            nc.sync.dma_start(masksf_sb[:], masksf[:])
            maskB16 = masksh_sb[:, 0:1]    # rows <= 126 (drop key col 1023)
            mask77A = masksh_sb[:, 1:2]    # rows <= 76 (start + evict rows of blk 1)

            ones_f = pres.tile([128, 1], F32, tag="ones_f")
            nc.vector.memset(ones_f[:], 1.0)
            ones = pres.tile([128, 1], F16, tag="ones")
            nc.vector.tensor_copy(ones[:], ones_f[:])

            # rope'd q/k in [d, t] layout; v in [t, d_local] layout
            qrT = [pres.tile([D, T], F16, tag=f"qrT{l}", name=f"qrT{l}") for l in range(HL)]
            krT = [pres.tile([D, T], F16, tag=f"krT{l}", name=f"krT{l}") for l in range(HL)]
            vt = [pres.tile([128, JC], F16, tag=f"vt{b}", name=f"vt{b}") for b in range(NB)]
            outT = [pres.tile([D, T], F16, tag=f"outT{l}", name=f"outT{l}") for l in range(HL)]

            F32R = mybir.dt.float32r

            def rope_pre(ps_ap, w):
                raw = ptmp.tile([128, w], F32R, tag="tmp")
                nc.scalar.copy(raw[:], ps_ap)
                return raw

            def rope_rot(raw, pool=None, tag="av"):
                rot = (pool or pav).tile([128, raw.shape[1]], F32, tag=tag)
                nc.tensor.matmul(rot[:], protT_sb[:], raw[:],
                                 start=True, stop=True)
                return rot

            def rope_post(raw, rot, dst_ap, c0, c1):
                w = c1 - c0
                t1 = ptmp.tile([128, w], F32, tag="tmp")
                nc.gpsimd.tensor_mul(t1[:], raw[:].bitcast(F32), cosT_sb[:, c0:c1])
                t2 = ptmp.tile([128, w], F32, tag="tmp")
                nc.vector.tensor_mul(t2[:], rot[:], sinTs_sb[:, c0:c1])
                nc.vector.tensor_add(dst_ap, t1[:], t2[:])

            # ---------------- phase K: kept keys (perm cols 0..1024), kc-outer ----
            psk = []
            for l in range(HL):
                pska = pps.tile([128, 512], F32, tag="ps", name=f"pska{l}")
                pskb = pps.tile([128, 512], F32, tag="ps", name=f"pskb{l}")
                psk.append((pska, pskb))
            for kc in range(KC):
                for l in range(HL):
                    wsl = wk_sb[:, kc * JC + l * 128: kc * JC + (l + 1) * 128]
                    nc.tensor.matmul(psk[l][0][:], wsl, hs_sb[kc][:, 0:512],
                                     start=(kc == 0), stop=(kc == KC - 1))
                    nc.tensor.matmul(psk[l][1][:], wsl, hs_sb[kc][:, 512:1024],
                                     start=(kc == 0), stop=(kc == KC - 1))
            kraw = []
            for l in range(HL):
                kraw.append((rope_pre(psk[l][0][:], 512),
                             rope_pre(psk[l][1][:], 512)))

            # ---------------- phase V (paired blocks, issued after q-tb0) ----------
            def v_phase():
                for bp in range(0, NB, 2):
                    if bp == 2:
                        for l in range(HL):
                            ra, rb = kraw[l]
                            rota = rope_rot(ra)
                            rope_post(ra, rota, krT[l][:, 0:512], 0, 512)
                            rotb = rope_rot(rb)
                            rope_post(rb, rotb, krT[l][:, 512:1024], 512, 1024)
                    psvA = pps.tile([128, JC], F32, tag="ps", name=f"psvA{bp}")
                    psvB = pps.tile([128, JC], F32, tag="ps", name=f"psvB{bp}")
                    for kc in range(KC):
                        nc.tensor.matmul(psvA[:], hs_sb[kc][:, bp * 128:(bp + 1) * 128],
                                         wv_sb[:, kc * JC:(kc + 1) * JC],
                                         start=(kc == 0), stop=(kc == KC - 1))
                        nc.tensor.matmul(psvB[:],
                                         hs_sb[kc][:, (bp + 1) * 128:(bp + 2) * 128],
                                         wv_sb[:, kc * JC:(kc + 1) * JC],
                                         start=(kc == 0), stop=(kc == KC - 1))
                    nc.scalar.copy(vt[bp][:], psvA[:])
                    nc.vector.tensor_copy(vt[bp + 1][:], psvB[:])
                    if bp + 1 == NB - 1:
                        # zero the non-key row (perm col 1023) so AV ignores it
                        nc.vector.tensor_scalar_mul(vt[bp + 1][:], vt[bp + 1][:],
                                                    masksf_sb[:, 0:1])

            # ---------------- phase Q + attention + pipelined o_proj ----------------
            def o_proj(tb):
                c0, c1 = TBS[tb]
                for ti in range(c0 // 128, c1 // 128):
                    last = (ti == 4 * TB - 1)
                    postg = ppost.tile([128, DM], F16, tag="post")
                    for mb in range(TB):
                        if mb < 2:
                            pso = pps.tile([128, 512], F32, tag="ps")
                        else:
                            pso = pav.tile([128, 512], F32, tag="av")
                        for l in range(HL):
                            nc.tensor.matmul(
                                pso[:], outT[l][:, ti * 128:(ti + 1) * 128],
                                wo_sb[:, l * DM + mb * 512: l * DM + (mb + 1) * 512],
                                start=(l == 0), stop=(l == HL - 1))
                        if mb % 2 == 0:
                            nc.scalar.copy(postg[:, mb * 512:(mb + 1) * 512], pso[:])
                        else:
                            nc.vector.tensor_copy(postg[:, mb * 512:(mb + 1) * 512],
                                                  pso[:])
                        if last:
                            eng = nc.scalar if mb % 2 == 0 else nc.sync
                            eng.dma_start(
                                po[ti * 128:(ti + 1) * 128,
                                   mb * 512:(mb + 1) * 512],
                                postg[:, mb * 512:(mb + 1) * 512])
                    if not last:
                        nc.sync.dma_start(po[ti * 128:(ti + 1) * 128, :], postg[:])

            v_phase()
            pend = {"fin": None}

            for tb in range(len(TBS)):
                c0, c1 = TBS[tb]
                w5 = c1 - c0
                ts5 = slice(c0, c1)
                qraw = []
                for l in range(HL):
                    psq = pps.tile([128, w5], F32, tag="ps")
                    for kc in range(KC):
                        nc.tensor.matmul(
                            psq[:], wq_sb[:, kc * JC + l * 128: kc * JC + (l + 1) * 128],
                            hs_sb[kc][:, ts5], start=(kc == 0), stop=(kc == KC - 1))
                    qraw.append(rope_pre(psq[:], w5))
                if pend["fin"] is not None:
                    pend["fin"]()
                    pend["fin"] = None
                for l in range(HL):
                    rot = rope_rot(qraw[l], pool=pps, tag="ps")
                    rope_post(qraw[l], rot, qrT[l][:, ts5], c0, c1)
                if tb > 1:
                    o_proj(tb - 2)

                cam = []  # per-head deferred CaM state
                psavl = []
                recipl = []
                for l in range(HL):
                    tail_it = (tb == len(TBS) - 1 and l == HL - 1)

                    def sc(b):
                        pst = pps.tile([128, w5], F32, tag="ps", name="pst")
                        nc.tensor.matmul(pst[:], krT[l][:, b * 128:(b + 1) * 128],
                                         qrT[l][:, ts5], start=True, stop=True)
                        e = pe16.tile([128, w5], F16, tag="e", name="e")
                        nc.scalar.activation(e[:], pst[:], AF.Exp)
                        if b == NB - 1:
                            # drop the non-key row (perm col 1023) from softmax
                            nc.vector.tensor_scalar_mul(e[:], e[:],
                                                        masksf_sb[:, 0:1])
                        E.append(e)

                    E = []
                    for b in range(4):
                        sc(b)
                    psav = pav.tile([128, w5], F32, tag="av")
                    if not tail_it:
                        # stagger: AV of block b-4 between later scores so the
                        # pst ring never waits on exp latency
                        for b in range(4, NB):
                            sc(b)
                            nc.tensor.matmul(psav[:],
                                             vt[b - 4][:, l * D:(l + 1) * D],
                                             E[b - 4][:],
                                             start=(b == 4), stop=False)
                    else:
                        for b in range(4, NB):
                            sc(b)
                    if tail_it:
                        # denominator first: its recip chain overlaps the AV MMs
                        psdn = ppsd.tile([1, w5], F32, tag="dn")
                        for b in range(NB):
                            nc.tensor.matmul(psdn[:], ones[:], E[b][:],
                                             start=(b == 0), stop=(b == NB - 1))
                        dnf = None
                        recip = prow.tile([1, w5], F32, tag="row512")
                        nc.vector.reciprocal(recip[:], psdn[0:1, :])
                        rbf = ptmp.tile([128, w5], F32, tag="tmp")
                        nc.gpsimd.partition_broadcast(rbf[:], recip[:])
                        for b in range(NB):
                            nc.tensor.matmul(psav[:], vt[b][:, l * D:(l + 1) * D],
                                             E[b][:],
                                             start=(b == 0), stop=(b == NB - 1))
                    else:
                        esum = pe16.tile([128, w5], F16, tag="e")
                        nc.vector.tensor_add(esum[:], E[0][:], E[1][:])
                        for b in range(2, NB):
                            nc.vector.tensor_add(esum[:], esum[:], E[b][:])
                        dnf = ptmp.tile([128, w5], F32, tag="tmp")
                        nc.gpsimd.partition_all_reduce(dnf[:], esum[:], channels=128,
                                                       reduce_op=bass_isa.ReduceOp.add)
                        rbf = ptmp.tile([128, w5], F32, tag="tmp")
                        nc.vector.reciprocal(rbf[:], dnf[:])
                        for b in range(4, NB):
                            nc.tensor.matmul(psav[:], vt[b][:, l * D:(l + 1) * D],
                                             E[b][:],
                                             start=False, stop=(b == NB - 1))
                    if tb != CAMTB:
                        nc.vector.tensor_mul(outT[l][:, ts5], psav[:], rbf[:])
                    psavl.append(psav)
                    recipl.append((rbf, dnf))

                    if tb == CAMTB:
                        # ---- CaM scalar chain (PE part deferred past attn l1) ----
                        # sum of E over start+evict rows at CaM cols (255..511)
                        pssA = ppsd.tile([1, 256], F32, tag="dn")
                        nc.tensor.matmul(pssA[:], ones[:], E[0][:, 255:511],
                                         start=True, stop=False)
                        nc.tensor.matmul(pssA[:], mask77A, E[1][:, 255:511],
                                         start=False, stop=True)
                        pssA_sb = prow.tile([1, 256], F32, tag="row256")
                        nc.scalar.copy(pssA_sb[:], pssA[:])
                        # E row of evict key (perm col 204 = block 1 row 76)
                        erow16 = prow.tile([1, 256], F16, tag="row256h")
                        nc.gpsimd.dma_start(erow16[:], E[1][76:77, 255:511])
                        erow = prow.tile([1, 256], F32, tag="row256")
                        nc.vector.tensor_copy(erow[:], erow16[:])
                        srec = prow.tile([1, 256], F32, tag="row256")
                        nc.vector.tensor_sub(srec[:], dnf[0:1, 255:511], pssA_sb[:])
                        # scalars at t = 2047 (perm col 1022 = within-slice 255)
                        r_last = rbf[0:1, 510:511]
                        num = prow.tile([1, 1], F32, tag="sc")
                        nc.vector.tensor_mul(num[:], erow[0:1, 255:256], r_last)
                        mean = prow.tile([1, 1], F32, tag="sc")
                        nc.vector.tensor_mul(mean[:], srec[0:1, 255:256], r_last)
                        nc.vector.tensor_scalar_mul(mean[:], mean[:], 1.0 / 818.0)
                        nc.vector.tensor_scalar_add(mean[:], mean[:], 1e-6)
                        um = prow.tile([1, 1], F32, tag="sc")
                        nc.vector.tensor_mul(um[:], u2_sb[0:1, l:l + 1], mean[:])
                        bern = prow.tile([1, 1], F32, tag="sc")
                        nc.vector.tensor_tensor(bern[:], um[:], num[:],
                                                mybir.AluOpType.is_lt)
                        bs = prow.tile([1, 1], F32, tag="sc")
                        nc.vector.tensor_scalar_mul(bs[:], bern[:], 1.0 / RB)
                        coef_f = prow.tile([1, 256], F32, tag="row256")
                        nc.vector.tensor_scalar_mul(coef_f[:], srec[:], bs[:])
                        coef = prow.tile([1, 256], F16, tag="row256h")
                        nc.vector.tensor_copy(coef[:], coef_f[:])
                        vrow = prow.tile([1, D], F16, tag="vrow")
                        nc.gpsimd.dma_start(vrow[:], vt[1][76:77, l * D:(l + 1) * D])
                        cam.append((coef, vrow))

                # normalize; at tb==CAMTB defer (incl. CaM rank-1) into next tb's
                # slot so the CaM scalar chain never head-blocks the PE queue
                if tb == CAMTB:
                    def fin(ts5=ts5, cam=cam, psavl=psavl, recipl=recipl):
                        for l in range(HL):
                            psav, (rbf, _dnf) = psavl[l], recipl[l]
                            coef, vrow = cam[l]
                            pscr = pps.tile([128, 256], F32, tag="ps")
                            nc.tensor.matmul(pscr[:], vrow[:], coef[:],
                                             start=True, stop=True)
                            nout = ptmp.tile([128, 512], F32, tag="tmp")
                            nc.vector.tensor_mul(nout[:], psav[:], rbf[:])
                            corr = ptmp.tile([128, 256], F32, tag="tmp")
                            nc.vector.tensor_mul(corr[:], pscr[:], rbf[:, 255:511])
                            nc.vector.tensor_add(nout[:, 255:511],
                                                 nout[:, 255:511], corr[:])
                            nc.vector.tensor_copy(outT[l][:, ts5], nout[:])
                    pend["fin"] = fin


            o_proj(len(TBS) - 2)
            o_proj(len(TBS) - 1)

    nc.compile()
    return nc


_NC_CACHE = None


def _get_nc():
    global _NC_CACHE
    if _NC_CACHE is None:
        _NC_CACHE = _build_nc()
    return _NC_CACHE


PERM = np.concatenate([np.arange(0, SB), np.arange(T - RB, T),
                       np.arange(SB, T - RB)])


def make_in_maps(hidden_states, Wq, Wk, Wv, Wo):
    hs = np.asarray(hidden_states, np.float32).reshape(T, DM)
    hs = np.nan_to_num(hs, nan=0.0, posinf=1e4, neginf=-1e4)
    hsT = np.ascontiguousarray(hs.T[:, PERM].astype(np.float16))
    Wq = np.asarray(Wq, np.float32)
    Wk = np.asarray(Wk, np.float32)
    Wv = np.asarray(Wv, np.float32)
    Wo = np.asarray(Wo, np.float32)

    inv_freq = 1.0 / (10000.0 ** (np.arange(0, D, 2, dtype=np.float32) / D))
    freqs = np.arange(T, dtype=np.float32)[:, None] * inv_freq[None, :]
    emb = np.concatenate([freqs, freqs], axis=-1)          # [T, D]
    cosT = np.ascontiguousarray(np.cos(emb).T[:, PERM].astype(np.float32))
    sinTs = np.ascontiguousarray(np.sin(emb).T[:, PERM].astype(np.float32))
    # rotate-half as a PE stationary: rot(x)[i] = -x[i+64] (i<64), x[i-64] (else)
    prot = np.zeros((D, D), np.float32)
    for i in range(64):
        prot[i + 64, i] = -1.0
        prot[i, i + 64] = 1.0

    import jax
    import jax.numpy as jnp
    u_full = np.asarray(
        jax.random.uniform(jax.random.key(42), (1, H), jnp.float32))

    maskh = np.zeros((128, 2), np.float16)
    maskh[:127, 0] = 1.0      # maskB16: drop row 127 of key block 7
    maskh[:77, 1] = 1.0       # mask77A: start+evict rows of key block 1
    maskf = np.zeros((128, 1), np.float32)
    maskf[:127, 0] = 1.0

    scale = 1.0 / np.sqrt(np.float32(D))

    def wlayout(wT):
        # wT: [DM, JC] -> SBUF layout [128, KC*JC]: [p, kc*JC + j]
        return np.ascontiguousarray(
            wT.reshape(KC, 128, JC).transpose(1, 0, 2).reshape(128, KC * JC)
            .astype(np.float16))

    in_maps = []
    for c in range(NCORES):
        js = slice(c * JC, (c + 1) * JC)
        woT = Wo[:, js].T                                  # [JC, DM]
        wo_l = np.ascontiguousarray(
            woT.reshape(HL, 128, DM).transpose(1, 0, 2).reshape(128, HL * DM)
            .astype(np.float16))
        in_maps.append({
            "hsT": hsT,
            "wq": wlayout(Wq[js, :].T * scale),
            "wk": wlayout(Wk[js, :].T),
            "wv": wlayout(Wv[js, :].T),
            "wo": wo_l,
            "cosT": cosT,
            "sinTs": sinTs,
            "protT": prot,
            "u2": np.ascontiguousarray(u_full[:, c * HL:(c + 1) * HL]),
            "masksh": maskh,
            "masksf": maskf,
        })
    return in_maps


def kernel(hidden_states, Wq, Wk, Wv, Wo):
    nc = _get_nc()
    in_maps = make_in_maps(hidden_states, Wq, Wk, Wv, Wo)
    res = bass_utils.run_bass_kernel_spmd(nc, in_maps,
                                          core_ids=list(range(NCORES)))
    acc = np.zeros((T, DM), np.float32)
    for c in range(NCORES):
        acc += res.results[c]["po"].astype(np.float32)
    out = np.empty((T, DM), np.float32)
    out[PERM] = acc                                       # undo token permutation
    out = np.nan_to_num(out, nan=0.0, posinf=1e4, neginf=-1e4)
    return out.reshape(1, T, DM)
